# revision 35
# baseline (speedup 1.0000x reference)
# MoE (8 experts, top-2) on 8 TRN2 NeuronCores — expert-parallel, tiered
# precision.
#
# Host (numpy): router matmul + softmax + top-2 (mirrors the jax reference
# fp32 arithmetic), then per-expert dispatch into TWO tiers:
#   tier A (fp16):  the expert's nbf largest-gate tokens — fp16 matmuls
#                   (same PE rate as bf16, 8x finer mantissa: its error is
#                   ~0.9e-3 vs bf16's 3.4e-3, freeing budget for more fp8).
#   tier C (fp8):   the remaining tokens (smallest gates) + zero padding up
#                   to nf8 — e4m3 matmuls in DoubleRow perf mode (measured
#                   2.0x PE rate vs bf16 at FD>=300).
# (nbf, nf8) are uniform across cores so the single SPMD program fits every
# expert; capacity padding rides in the half-cost fp8 tier. Putting only
# low-gate tokens in fp8 keeps the quantization error a small fraction of
# the output norm (measured 1.83e-2 vs the 2e-2 gate; fp16-only is ~1e-3;
# the numpy e4m3 simulation in the dev harness predicts device error to
# ~1e-5, so the tier split was tuned against it).
#
# Device (per core, expert e), per token block:
#   mm1: hT = gelu(W1^T @ xT + b1)   (A: fp16, 8 K-chunks; C: e4m3 DoubleRow,
#        4 K-chunk-pairs, PSUM descaled 1/8192 inside the ACT op)
#   mm2: yT = (W2^T @ hT) * gate     (A: fp16; C: e4m3 DoubleRow with the
#        1/1024 W2 scale folded into the gate values — free)
# Both matmuls keep tokens on the moving/free axis so work scales with the
# exact token count. Gates and outputs are fp16 (halves the drain DMA).
#
# Weights are streamed per block (W1 slices, fp8 W2 d-slices) except fp16 W2
# which stays resident as 8 d-slices; SBUF peak ~26 MB, total DMA ~69
# MB/core — hidden under the ~390 us of PE time. Stream issue order is
# tuned so every tensor lands just before its first PE use (the in-order
# engine queues stall head-of-line otherwise).
#
# Shapes hardcoded for B=4, S=2048, D=1024, H=4096, E=8 (spec); the builder
# is parametric in the block lists (known only after routing).

import numpy as np
import ml_dtypes

NUM_EXPERTS = 8
TOP_K = 2
P = 128          # SBUF partitions
TB = 512         # token block (matmul moving free size)

# Tier sizes (tokens per expert): tuned offline on the seed-0 routing
# (counts 1932..2182, mean 2048). nbf + nf8 must cover the max expert load;
# the builder re-derives nf8 at runtime if routing ever exceeds it.
NBF_TARGET = 1376
NF8_TARGET = 816

# fp8 scales (powers of two; e4m3 max 240, scaled data max ~87)
SX = 16.0        # x
SW1 = 512.0      # W1  -> psum1 = 8192 * (x @ W1), descaled in the gelu ACT
SW2 = 1024.0     # W2  -> psum2 = 1024 * (h @ W2), folded into device gates

_program_cache = {}


def _split_blocks(n, mult16):
    """Split n tokens into blocks of <=TB, remainder last (block 0 must be
    full so its compute covers the next block's weight stream). For fp8
    (mult16) every block size must be a multiple of 16 (DoubleRow pair-slice
    stride) and at least 128 (DoubleRow is a net loss below FD=128), so a
    small remainder is rebalanced across the last two blocks."""
    if n == 0:
        return []
    sizes = [TB] * (n // TB)
    rem = n % TB
    if rem:
        sizes.append(rem)
    if mult16:
        assert n % 16 == 0
        if len(sizes) >= 2:
            # near-equal split: keeps every fp8 block >=256 (full DoubleRow
            # win) and shrinks the largest fp8 tile, which trims the h8/x8
            # SBUF footprint
            k = len(sizes)
            base = (n // k) // 16 * 16
            sizes = [base] * (k - 1) + [n - base * (k - 1)]
            assert sizes[-1] <= TB
    else:
        if len(sizes) >= 2 and sizes[-1] < 256:
            pair = sizes[-2] + sizes[-1]
            sizes[-2:] = [pair // 2, pair - pair // 2]
    return sizes


def _build_program(sizes_a, sizes_c, D, H):
    import concourse.mybir as mybir
    import concourse.tile as tile
    from concourse import bacc

    fh = mybir.dt.float16
    f8 = mybir.dt.float8e4
    f32 = mybir.dt.float32
    Gelu = mybir.ActivationFunctionType.Gelu_apprx_tanh
    DR = mybir.MatmulPerfMode.DoubleRow

    KD = D // P      # mm1 contraction chunks / mm2 output row chunks (8)
    KH = H // P      # mm1 output chunks / mm2 contraction chunks (32)
    MJ = 16          # W1 column-slice count (2 m-tiles per slice)
    HJ = H // MJ

    nbf = sum(sizes_a)
    nf8 = sum(sizes_c)
    ntot = nbf + nf8

    # Host-packed dram layouts (partition dim first, per-partition contiguous
    # runs >= 1KB so DMAs stream at full bandwidth):
    #   xtb : [P, KD*nbf] fp16    block-packed tier-A tokens
    #   xt8 : [P, KD*nf8] e4m3    block-packed tier-C tokens (x*16)
    #   w1b : [P, MJ, KD, HJ] fp16
    #   w18 : [P, MJ, KD, HJ] e4m3 (W1*512)
    #   w2b : [P, KD, KH, P]  fp16 (resident, d-major slices)
    #   w28 : [P, KD, KH, P]  e4m3 (W2*1024, d-major slices)
    #   gb  : [P, ntot] fp16      gates, tier-C segment pre-divided by 1024
    #   b1t : [P, KH] f32
    #   ytr : [D, ntot] fp16      transposed output, tier order A then C
    nc = bacc.Bacc(None, target_bir_lowering=False, debug=False)
    xtb = nc.declare_dram_parameter("xtb", [P, KD * max(nbf, 1)], fh, isOutput=False).ap()
    xt8 = nc.declare_dram_parameter("xt8", [P, KD * max(nf8, 1)], f8, isOutput=False).ap()
    w1b = nc.declare_dram_parameter("w1b", [P, MJ, KD, HJ], fh, isOutput=False).ap()
    w18 = nc.declare_dram_parameter("w18", [P, MJ, KD, HJ], f8, isOutput=False).ap()
    w2b = nc.declare_dram_parameter("w2b", [P, KD, KH, P], fh, isOutput=False).ap()
    w28 = nc.declare_dram_parameter("w28", [P, KD, KH, P], f8, isOutput=False).ap()
    gb = nc.declare_dram_parameter("gb", [P, ntot], fh, isOutput=False).ap()
    b1t = nc.declare_dram_parameter("b1t", [P, KH], f32, isOutput=False).ap()
    ytr = nc.declare_dram_parameter("ytr", [D, ntot], fh, isOutput=True).ap()
    # [P, KD, ntot] view of the [D, ntot] output for one-DMA-per-block stores
    ytr3 = ytr.rearrange("(d p) n -> p d n", p=P)

    # per-block metadata: (tbs, isf8, t0seg=offset within own tier, t0=global)
    blocks = []
    t0a = t0c = t0 = 0
    for tbs in sizes_a:
        blocks.append((tbs, False, t0a, t0))
        t0a += tbs
        t0 += tbs
    for tbs in sizes_c:
        blocks.append((tbs, True, t0c, t0))
        t0c += tbs
        t0 += tbs
    nblk = len(blocks)

    with tile.TileContext(nc) as tc:
        with (
            tc.tile_pool(name="wres", bufs=1) as wres,
            tc.tile_pool(name="wstr", bufs=4) as wstr,
            tc.tile_pool(name="data", bufs=2) as datap,
            tc.tile_pool(name="psum", bufs=4, space="PSUM") as psump,
        ):
            # merged pools (fewer pools -> shorter entry/exit barrier
            # handshake); rotation depth is set per tag via bufs=
            w1sp = w28p = wstr
            xpool = hpool = ypool = gbp = datap
            php = pyp = psump
            b1_sb = wres.tile([P, KH], f32, tag="b1sb")
            # resident bf16 W2 as 8 d-major slices: the tile scheduler
            # hoists a few mm2 matmuls high into the in-order PE queue, so
            # the first d-slice must land early (a monolithic 8.4 MB DMA
            # stalled the PE ~10-15 us).
            w2b_sl = [
                wres.tile([P, KH, P], fh, tag=f"w2bd{d}", name=f"w2bd{d}")
                for d in range(KD)
            ]

            # per-block input tiles, possibly issued one block ahead
            xt_tiles = {}
            gb_tiles = {}
            w1_tiles = {b: [None] * MJ for b in range(nblk)}

            def issue_x(b):
                tbs, isf8, t0seg, t0b = blocks[b]
                if isf8:
                    xt_blk = xpool.tile([P, KD, tbs], f8, tag="x8")
                    src, base = xt8, KD * t0seg
                else:
                    xt_blk = xpool.tile([P, KD, tbs], fh, tag="xt")
                    src, base = xtb, KD * t0seg
                if b == 0:
                    # chunk 0 alone first (the very first matmul's only x
                    # dependency — a 128 KB transfer that lands fast), then
                    # the rest split across queues
                    nc.sync.dma_start(
                        xt_blk[:, 0, :], src[:, base:base + tbs]
                    )
                    for k in range(1, KD):
                        nc.sync.dma_start(
                            xt_blk[:, k, :],
                            src[:, base + k * tbs:base + (k + 1) * tbs],
                        )
                else:
                    nc.sync.dma_start(
                        xt_blk,
                        src[:, base:base + KD * tbs].rearrange(
                            "p (k c) -> p k c", k=KD
                        ),
                    )
                xt_tiles[b] = xt_blk
                gb_sb = gbp.tile([P, tbs], fh, tag="gb")
                nc.sync.dma_start(gb_sb, gb[:, t0b:t0b + tbs])
                gb_tiles[b] = gb_sb

            def issue_w1(b, js):
                _, isf8, _, _ = blocks[b]
                wsrc, wdt, wtag = (w18, f8, "w18s") if isf8 else (w1b, fh, "w1bs")
                for j in js:
                    if w1_tiles[b][j] is not None:
                        continue
                    ws = w1sp.tile([P, KD, HJ], wdt, tag=wtag)
                    nc.sync.dma_start(ws, wsrc[:, j, :, :])
                    w1_tiles[b][j] = ws

            # PE pstate warmup: the engine runs its first ~3 us at a reduced
            # p-state (first real m-tile measured 427-634 ns/matmul instead
            # of 216). Zero matmuls on memset tiles ramp it up during the
            # startup DMA window; they retire before the first x/W1 slice
            # lands (~11 us), so they never delay real work.
            warm_l = wres.tile([P, P], fh, tag="warm_l")
            warm_r = wres.tile([P, TB], fh, tag="warm_r")
            nc.vector.memset(warm_l, 0.0)
            nc.vector.memset(warm_r, 0.0)
            for _ in range(12):
                pw = php.tile([P, TB], f32, tag="ph")
                nc.tensor.matmul(pw, warm_l, warm_r, start=True, stop=True)

            issue_w1(0, range(0, 1))
            issue_x(0)
            nc.sync.dma_start(b1_sb, b1t)
            issue_w1(0, range(1, 4))
            # d0 early: the scheduler hoists a few mm2-d0 matmuls between
            # mm1 m-tiles ~6-7, so d0 must land by ~25 us. d1..d7 are only
            # needed at the real mm2 (~105 us) — after the W1 stream.
            nc.sync.dma_start(w2b_sl[0], w2b[:, 0, :, :])
            issue_w1(0, range(4, MJ))
            for d in range(1, KD):
                nc.sync.dma_start(w2b_sl[d], w2b[:, d, :, :])

            for b, (tbs, isf8, t0seg, t0b) in enumerate(blocks):
                # finish this block's input streams (x/gb/first W1 slices
                # were issued during the previous block). For fp8 blocks the
                # W2 d-slices are interleaved into the W1 stream by need
                # time (W1 slice j feeds m-tiles 2j at ~j*1.05 us; W2 slice
                # d feeds mm2 at ~mm1_end + d*2.6 us) — issuing all W1
                # first made mm2's first d-slices arrive late (~3.5 us PE
                # stall per fp8 block).
                w28_sl = []
                if isf8:
                    def issue_w28(dds):
                        for dd in dds:
                            w2s = w28p.tile([P, KH, P], f8, tag="w28s", bufs=8)
                            nc.sync.dma_start(w2s, w28[:, dd, :, :])
                            w28_sl.append(w2s)
                    issue_w1(b, range(0, 10))
                    issue_w28(range(0, 2))
                    issue_w1(b, range(10, MJ))
                    issue_w28(range(2, KD))
                else:
                    issue_w1(b, range(MJ))
                # prefetch the next block's x, gates, and first W1 slices so
                # the A->C phase switch doesn't wait behind this block's
                # 8.4 MB W1 stream
                if b + 1 < nblk:
                    issue_x(b + 1)
                    issue_w1(b + 1, range(0, 3))

                xt_blk = xt_tiles.pop(b)
                gb_sb = gb_tiles.pop(b)
                w1_sl = w1_tiles.pop(b)

                # --- mm1: hT[m] = gelu(W1_chunk^T @ xT + b1) -> [P, tbs]
                hT = hpool.tile([P, KH, tbs], f8 if isf8 else fh,
                                tag="h8" if isf8 else "hT", bufs=1)
                for m in range(KH):
                    ph = php.tile([P, tbs], f32, tag="ph")
                    mj, mo = divmod(m, HJ // P)
                    if isf8:
                        for k in range(KD // 2):
                            nc.tensor.matmul(
                                ph,
                                w1_sl[mj][:, 2 * k:2 * k + 2, mo * P:(mo + 1) * P],
                                xt_blk[:, 2 * k:2 * k + 2, :],
                                start=(k == 0),
                                stop=(k == KD // 2 - 1),
                                perf_mode=DR,
                            )
                        nc.scalar.activation(
                            hT[:, m, :], ph, Gelu,
                            bias=b1_sb[:, m:m + 1], scale=1.0 / (SX * SW1),
                        )
                    else:
                        for k in range(KD):
                            nc.tensor.matmul(
                                ph,
                                w1_sl[mj][:, k, mo * P:(mo + 1) * P],
                                xt_blk[:, k, :],
                                start=(k == 0),
                                stop=(k == KD - 1),
                            )
                        nc.scalar.activation(
                            hT[:, m, :], ph, Gelu, bias=b1_sb[:, m:m + 1]
                        )

                # --- mm2: yT[d] = (W2_chunk^T @ hT) * gate, one output DMA
                # per block (8 small stores per block cost ~0.6 us of SP
                # issue each and serialized the drain tail)
                yt_blk = ypool.tile([P, KD, tbs], fh, tag="yt")
                for d in range(KD):
                    pyT = pyp.tile([P, tbs], f32, tag="py")
                    if isf8:
                        for k in range(KH // 2):
                            nc.tensor.matmul(
                                pyT,
                                w28_sl[d][:, 2 * k:2 * k + 2, :],
                                hT[:, 2 * k:2 * k + 2, :],
                                start=(k == 0),
                                stop=(k == KH // 2 - 1),
                                perf_mode=DR,
                            )
                    else:
                        for k in range(KH):
                            nc.tensor.matmul(
                                pyT,
                                w2b_sl[d][:, k, :],
                                hT[:, k, :],
                                start=(k == 0),
                                stop=(k == KH - 1),
                            )
                    # fused PSUM evacuation + gate broadcast multiply on DVE
                    nc.vector.tensor_mul(yt_blk[:, d, :], pyT, gb_sb)
                    if b == nblk - 1:
                        # last block: store each d-slice as its own DMA so
                        # the drain pipelines across queues behind the PE
                        # instead of one serial descriptor chain at the end
                        nc.sync.dma_start(
                            ytr3[:, d, t0b:t0b + tbs], yt_blk[:, d, :]
                        )
                if b != nblk - 1:
                    nc.sync.dma_start(ytr3[:, :, t0b:t0b + tbs], yt_blk)
    nc.compile()
    return nc


def _ensure_trace_hooks():
    # bass_utils' trace path (taken when BASS_TRACE=1 is set externally)
    # imports antenv.axon_hooks, which this image lacks. Shim it (and the
    # artifact upload, which needs a bucket) only when missing, so tracing
    # degrades gracefully instead of crashing.
    import sys
    import types

    try:
        import antenv.axon_hooks  # noqa: F401
        return
    except ImportError:
        pass
    try:
        import antenv

        mod = types.ModuleType("antenv.axon_hooks")
        state = {"hook": None}
        mod.set_axon_ntff_profile_hook = lambda h: state.__setitem__("hook", h)
        mod.get_axon_ntff_profile_hook = lambda: state["hook"]
        sys.modules["antenv.axon_hooks"] = mod
        antenv.axon_hooks = mod
        try:
            from trn_agent_boot.trn_boot import _ntff_profile_via_ctypes

            mod.set_axon_ntff_profile_hook(
                _ntff_profile_via_ctypes("/opt/axon/libaxon_pjrt.so")
            )
            import concourse.bass_utils as _bu

            _orig_upload = _bu.upload_artifacts

            def _safe_upload(tmpdir):
                try:
                    return _orig_upload(tmpdir)
                except Exception:
                    return f"local:{tmpdir}"

            _bu.upload_artifacts = _safe_upload
        except Exception:
            pass
    except Exception:
        pass


def kernel(x, Wr, W1, b1, W2, b2):
    _ensure_trace_hooks()
    from concourse.bass_utils import run_bass_kernel_spmd

    f16 = np.float16
    e4m3 = ml_dtypes.float8_e4m3
    B, S, D = x.shape
    E, _, H = W1.shape
    N = B * S
    KD = D // P
    MJ = 16
    HJ = H // MJ
    xm = np.ascontiguousarray(x.reshape(N, D), dtype=np.float32)

    # --- host router (mirrors reference fp32 arithmetic; softmax is
    # monotonic so top-k on probs == top-k on logits, ties broken by index)
    logits = xm @ Wr
    mx = logits.max(axis=1, keepdims=True)
    ex = np.exp(logits - mx)
    probs = ex / ex.sum(axis=1, keepdims=True)
    top_i = np.argsort(-probs, axis=1, kind="stable")[:, :TOP_K]

    idx = [np.where((top_i == e).any(axis=1))[0] for e in range(E)]
    counts = np.array([len(i) for i in idx])
    cmax = int(counts.max())

    nbf = NBF_TARGET
    nf8 = max(NF8_TARGET, -((nbf - cmax) // 16) * 16)  # ceil16(cmax-nbf)
    sizes_a = _split_blocks(nbf, mult16=False)
    sizes_c = _split_blocks(nf8, mult16=True)
    ntot = nbf + nf8

    # --- dispatch: per expert, sort tokens by gate descending; largest nbf
    # gates -> tier A (bf16), rest -> tier C (fp8) + zero padding.
    xT = np.ascontiguousarray(xm.T).astype(f16)               # [D, N] fp16
    xT8 = np.ascontiguousarray((xm.T * SX)).astype(e4m3)      # [D, N] e4m3
    in_maps = []
    tok_a, tok_c = [], []
    for e in range(E):
        ge = probs[idx[e], e]
        order = np.argsort(-ge, kind="stable")
        ta = idx[e][order[:nbf]]
        tc = idx[e][order[nbf:]]
        tok_a.append(ta)
        tok_c.append(tc)

        def pack_blocks(src, toks, ncap, sizes, dt):
            # [P, KD*ncap] block-packed SBUF layout
            xte = np.zeros((D, ncap), dtype=dt)
            xte[:, :len(toks)] = src[:, toks]
            xte3 = xte.reshape(KD, P, ncap).transpose(1, 0, 2)
            t0 = 0
            chunks = []
            for tbs in sizes:
                chunks.append(xte3[:, :, t0:t0 + tbs].reshape(P, -1))
                t0 += tbs
            return np.ascontiguousarray(np.concatenate(chunks, axis=1))

        w1f = np.asarray(W1[e], dtype=np.float32)
        w2f = np.asarray(W2[e], dtype=np.float32)
        gfull = np.zeros((ntot,), dtype=np.float32)
        gfull[:len(ta)] = probs[ta, e]
        gfull[nbf:nbf + len(tc)] = probs[tc, e] / SW2
        in_maps.append({
            "xtb": pack_blocks(xT, ta, nbf, sizes_a, f16),
            "xt8": pack_blocks(xT8, tc, nf8, sizes_c, e4m3),
            "w1b": np.ascontiguousarray(
                w1f.astype(f16).reshape(KD, P, MJ, HJ).transpose(1, 2, 0, 3)
            ),
            "w18": np.ascontiguousarray(
                (w1f * SW1).astype(e4m3).reshape(KD, P, MJ, HJ).transpose(1, 2, 0, 3)
            ),
            "w2b": np.ascontiguousarray(
                w2f.astype(f16).reshape(H // P, P, KD, P).transpose(1, 2, 0, 3)
            ),
            "w28": np.ascontiguousarray(
                (w2f * SW2).astype(e4m3).reshape(H // P, P, KD, P).transpose(1, 2, 0, 3)
            ),
            "gb": np.ascontiguousarray(np.broadcast_to(gfull, (P, ntot)).astype(f16)),
            "b1t": np.ascontiguousarray(
                np.asarray(b1[e], dtype=np.float32).reshape(H // P, P).T
            ),
        })

    key = (tuple(sizes_a), tuple(sizes_c), D, H)
    if key not in _program_cache:
        _program_cache[key] = _build_program(sizes_a, sizes_c, D, H)
    nc = _program_cache[key]

    res = run_bass_kernel_spmd(nc, in_maps, core_ids=list(range(NUM_EXPERTS)))

    # --- combine: transpose each expert's [D, n] block and scatter-add
    # (indices unique per expert)
    out = np.zeros((N, D), dtype=np.float32)
    b2f = np.asarray(b2, dtype=np.float32)
    for e in range(E):
        ytr = np.asarray(res.results[e]["ytr"]).astype(np.float32)
        for toks, seg0 in ((tok_a[e], 0), (tok_c[e], nbf)):
            if not len(toks):
                continue
            ye = np.ascontiguousarray(ytr[:, seg0:seg0 + len(toks)].T)
            if b2f[e].any():
                ye = ye + probs[toks, e][:, None] * b2f[e]
            out[toks] += ye
    return out.reshape(B, S, D)


# revision 36
# speedup vs baseline: 1.0047x; 1.0047x over previous
# MoE (8 experts, top-2) on 8 TRN2 NeuronCores — expert-parallel, tiered
# precision.
#
# Host (numpy): router matmul + softmax + top-2 (mirrors the jax reference
# fp32 arithmetic), then per-expert dispatch into TWO tiers:
#   tier A (fp16):  the expert's nbf largest-gate tokens — fp16 matmuls
#                   (same PE rate as bf16, 8x finer mantissa: its error is
#                   ~0.9e-3 vs bf16's 3.4e-3, freeing budget for more fp8).
#   tier C (fp8):   the remaining tokens (smallest gates) + zero padding up
#                   to nf8 — e4m3 matmuls in DoubleRow perf mode (measured
#                   2.0x PE rate vs bf16 at FD>=300).
# (nbf, nf8) are uniform across cores so the single SPMD program fits every
# expert; capacity padding rides in the half-cost fp8 tier. Putting only
# low-gate tokens in fp8 keeps the quantization error a small fraction of
# the output norm (measured 1.83e-2 vs the 2e-2 gate; fp16-only is ~1e-3;
# the numpy e4m3 simulation in the dev harness predicts device error to
# ~1e-5, so the tier split was tuned against it).
#
# Device (per core, expert e), per token block:
#   mm1: hT = gelu(W1^T @ xT + b1)   (A: fp16, 8 K-chunks; C: e4m3 DoubleRow,
#        4 K-chunk-pairs, PSUM descaled 1/8192 inside the ACT op)
#   mm2: yT = (W2^T @ hT) * gate     (A: fp16; C: e4m3 DoubleRow with the
#        1/1024 W2 scale folded into the gate values — free)
# Both matmuls keep tokens on the moving/free axis so work scales with the
# exact token count. Gates and outputs are fp16 (halves the drain DMA).
#
# Weights are streamed per block (W1 slices, fp8 W2 d-slices) except fp16 W2
# which stays resident as 8 d-slices; SBUF peak ~26 MB, total DMA ~69
# MB/core — hidden under the ~390 us of PE time. Stream issue order is
# tuned so every tensor lands just before its first PE use (the in-order
# engine queues stall head-of-line otherwise).
#
# Shapes hardcoded for B=4, S=2048, D=1024, H=4096, E=8 (spec); the builder
# is parametric in the block lists (known only after routing).

import numpy as np
import ml_dtypes

NUM_EXPERTS = 8
TOP_K = 2
P = 128          # SBUF partitions
TB = 512         # token block (matmul moving free size)

# Tier sizes (tokens per expert): tuned offline on the seed-0 routing
# (counts 1932..2182, mean 2048). nbf + nf8 must cover the max expert load;
# the builder re-derives nf8 at runtime if routing ever exceeds it.
NBF_TARGET = 1344
NF8_TARGET = 848

# fp8 scales (powers of two; e4m3 max 240, scaled data max ~87)
SX = 16.0        # x
SW1 = 512.0      # W1  -> psum1 = 8192 * (x @ W1), descaled in the gelu ACT
SW2 = 1024.0     # W2  -> psum2 = 1024 * (h @ W2), folded into device gates

_program_cache = {}


def _split_blocks(n, mult16):
    """Split n tokens into blocks of <=TB, remainder last (block 0 must be
    full so its compute covers the next block's weight stream). For fp8
    (mult16) every block size must be a multiple of 16 (DoubleRow pair-slice
    stride) and at least 128 (DoubleRow is a net loss below FD=128), so a
    small remainder is rebalanced across the last two blocks."""
    if n == 0:
        return []
    sizes = [TB] * (n // TB)
    rem = n % TB
    if rem:
        sizes.append(rem)
    if mult16:
        assert n % 16 == 0
        if len(sizes) >= 2:
            # near-equal split: keeps every fp8 block >=256 (full DoubleRow
            # win) and shrinks the largest fp8 tile, which trims the h8/x8
            # SBUF footprint
            k = len(sizes)
            base = (n // k) // 16 * 16
            sizes = [base] * (k - 1) + [n - base * (k - 1)]
            assert sizes[-1] <= TB
    else:
        if len(sizes) >= 2 and sizes[-1] < 256:
            pair = sizes[-2] + sizes[-1]
            sizes[-2:] = [pair // 2, pair - pair // 2]
    return sizes


def _build_program(sizes_a, sizes_c, D, H):
    import concourse.mybir as mybir
    import concourse.tile as tile
    from concourse import bacc

    fh = mybir.dt.float16
    f8 = mybir.dt.float8e4
    f32 = mybir.dt.float32
    Gelu = mybir.ActivationFunctionType.Gelu_apprx_tanh
    DR = mybir.MatmulPerfMode.DoubleRow

    KD = D // P      # mm1 contraction chunks / mm2 output row chunks (8)
    KH = H // P      # mm1 output chunks / mm2 contraction chunks (32)
    MJ = 16          # W1 column-slice count (2 m-tiles per slice)
    HJ = H // MJ

    nbf = sum(sizes_a)
    nf8 = sum(sizes_c)
    ntot = nbf + nf8

    # Host-packed dram layouts (partition dim first, per-partition contiguous
    # runs >= 1KB so DMAs stream at full bandwidth):
    #   xtb : [P, KD*nbf] fp16    block-packed tier-A tokens
    #   xt8 : [P, KD*nf8] e4m3    block-packed tier-C tokens (x*16)
    #   w1b : [P, MJ, KD, HJ] fp16
    #   w18 : [P, MJ, KD, HJ] e4m3 (W1*512)
    #   w2b : [P, KD, KH, P]  fp16 (resident, d-major slices)
    #   w28 : [P, KD, KH, P]  e4m3 (W2*1024, d-major slices)
    #   gb  : [P, ntot] fp16      gates, tier-C segment pre-divided by 1024
    #   b1t : [P, KH] f32
    #   ytr : [D, ntot] fp16      transposed output, tier order A then C
    nc = bacc.Bacc(None, target_bir_lowering=False, debug=False)
    xtb = nc.declare_dram_parameter("xtb", [P, KD * max(nbf, 1)], fh, isOutput=False).ap()
    xt8 = nc.declare_dram_parameter("xt8", [P, KD * max(nf8, 1)], f8, isOutput=False).ap()
    w1b = nc.declare_dram_parameter("w1b", [P, MJ, KD, HJ], fh, isOutput=False).ap()
    w18 = nc.declare_dram_parameter("w18", [P, MJ, KD, HJ], f8, isOutput=False).ap()
    w2b = nc.declare_dram_parameter("w2b", [P, KD, KH, P], fh, isOutput=False).ap()
    w28 = nc.declare_dram_parameter("w28", [P, KD, KH, P], f8, isOutput=False).ap()
    gb = nc.declare_dram_parameter("gb", [P, ntot], fh, isOutput=False).ap()
    b1t = nc.declare_dram_parameter("b1t", [P, KH], f32, isOutput=False).ap()
    ytr = nc.declare_dram_parameter("ytr", [D, ntot], fh, isOutput=True).ap()
    # [P, KD, ntot] view of the [D, ntot] output for one-DMA-per-block stores
    ytr3 = ytr.rearrange("(d p) n -> p d n", p=P)

    # per-block metadata: (tbs, isf8, t0seg=offset within own tier, t0=global)
    blocks = []
    t0a = t0c = t0 = 0
    for tbs in sizes_a:
        blocks.append((tbs, False, t0a, t0))
        t0a += tbs
        t0 += tbs
    for tbs in sizes_c:
        blocks.append((tbs, True, t0c, t0))
        t0c += tbs
        t0 += tbs
    nblk = len(blocks)

    with tile.TileContext(nc) as tc:
        with (
            tc.tile_pool(name="wres", bufs=1) as wres,
            tc.tile_pool(name="wstr", bufs=4) as wstr,
            tc.tile_pool(name="data", bufs=2) as datap,
            tc.tile_pool(name="psum", bufs=4, space="PSUM") as psump,
        ):
            # merged pools (fewer pools -> shorter entry/exit barrier
            # handshake); rotation depth is set per tag via bufs=
            w1sp = w28p = wstr
            xpool = hpool = ypool = gbp = datap
            php = pyp = psump
            b1_sb = wres.tile([P, KH], f32, tag="b1sb")
            # resident bf16 W2 as 8 d-major slices: the tile scheduler
            # hoists a few mm2 matmuls high into the in-order PE queue, so
            # the first d-slice must land early (a monolithic 8.4 MB DMA
            # stalled the PE ~10-15 us).
            w2b_sl = [
                wres.tile([P, KH, P], fh, tag=f"w2bd{d}", name=f"w2bd{d}")
                for d in range(KD)
            ]

            # per-block input tiles, possibly issued one block ahead
            xt_tiles = {}
            gb_tiles = {}
            w1_tiles = {b: [None] * MJ for b in range(nblk)}

            def issue_x(b):
                tbs, isf8, t0seg, t0b = blocks[b]
                if isf8:
                    xt_blk = xpool.tile([P, KD, tbs], f8, tag="x8")
                    src, base = xt8, KD * t0seg
                else:
                    xt_blk = xpool.tile([P, KD, tbs], fh, tag="xt")
                    src, base = xtb, KD * t0seg
                if b == 0:
                    # chunk 0 alone first (the very first matmul's only x
                    # dependency — a 128 KB transfer that lands fast), then
                    # the rest split across queues
                    nc.sync.dma_start(
                        xt_blk[:, 0, :], src[:, base:base + tbs]
                    )
                    for k in range(1, KD):
                        nc.sync.dma_start(
                            xt_blk[:, k, :],
                            src[:, base + k * tbs:base + (k + 1) * tbs],
                        )
                else:
                    nc.sync.dma_start(
                        xt_blk,
                        src[:, base:base + KD * tbs].rearrange(
                            "p (k c) -> p k c", k=KD
                        ),
                    )
                xt_tiles[b] = xt_blk
                gb_sb = gbp.tile([P, tbs], fh, tag="gb")
                nc.sync.dma_start(gb_sb, gb[:, t0b:t0b + tbs])
                gb_tiles[b] = gb_sb

            def issue_w1(b, js):
                _, isf8, _, _ = blocks[b]
                wsrc, wdt, wtag = (w18, f8, "w18s") if isf8 else (w1b, fh, "w1bs")
                for j in js:
                    if w1_tiles[b][j] is not None:
                        continue
                    ws = w1sp.tile([P, KD, HJ], wdt, tag=wtag)
                    nc.sync.dma_start(ws, wsrc[:, j, :, :])
                    w1_tiles[b][j] = ws

            # PE pstate warmup: the engine runs its first ~3 us at a reduced
            # p-state (first real m-tile measured 427-634 ns/matmul instead
            # of 216). Zero matmuls on memset tiles ramp it up during the
            # startup DMA window; they retire before the first x/W1 slice
            # lands (~11 us), so they never delay real work.
            warm_l = wres.tile([P, P], fh, tag="warm_l")
            warm_r = wres.tile([P, TB], fh, tag="warm_r")
            nc.vector.memset(warm_l, 0.0)
            nc.vector.memset(warm_r, 0.0)
            for _ in range(12):
                pw = php.tile([P, TB], f32, tag="ph")
                nc.tensor.matmul(pw, warm_l, warm_r, start=True, stop=True)

            issue_w1(0, range(0, 1))
            issue_x(0)
            nc.sync.dma_start(b1_sb, b1t)
            issue_w1(0, range(1, 4))
            # d0 early: the scheduler hoists a few mm2-d0 matmuls between
            # mm1 m-tiles ~6-7, so d0 must land by ~25 us. d1..d7 are only
            # needed at the real mm2 (~105 us) — after the W1 stream.
            nc.sync.dma_start(w2b_sl[0], w2b[:, 0, :, :])
            issue_w1(0, range(4, MJ))
            for d in range(1, KD):
                nc.sync.dma_start(w2b_sl[d], w2b[:, d, :, :])

            for b, (tbs, isf8, t0seg, t0b) in enumerate(blocks):
                # finish this block's input streams (x/gb/first W1 slices
                # were issued during the previous block). For fp8 blocks the
                # W2 d-slices are interleaved into the W1 stream by need
                # time (W1 slice j feeds m-tiles 2j at ~j*1.05 us; W2 slice
                # d feeds mm2 at ~mm1_end + d*2.6 us) — issuing all W1
                # first made mm2's first d-slices arrive late (~3.5 us PE
                # stall per fp8 block).
                w28_sl = []
                if isf8:
                    def issue_w28(dds):
                        for dd in dds:
                            w2s = w28p.tile([P, KH, P], f8, tag="w28s", bufs=8)
                            nc.sync.dma_start(w2s, w28[:, dd, :, :])
                            w28_sl.append(w2s)
                    issue_w1(b, range(0, 10))
                    issue_w28(range(0, 2))
                    issue_w1(b, range(10, MJ))
                    issue_w28(range(2, KD))
                else:
                    issue_w1(b, range(MJ))
                # prefetch the next block's x, gates, and first W1 slices so
                # the A->C phase switch doesn't wait behind this block's
                # 8.4 MB W1 stream
                if b + 1 < nblk:
                    issue_x(b + 1)
                    issue_w1(b + 1, range(0, 3))

                xt_blk = xt_tiles.pop(b)
                gb_sb = gb_tiles.pop(b)
                w1_sl = w1_tiles.pop(b)

                # --- mm1: hT[m] = gelu(W1_chunk^T @ xT + b1) -> [P, tbs]
                hT = hpool.tile([P, KH, tbs], f8 if isf8 else fh,
                                tag="h8" if isf8 else "hT", bufs=1)
                for m in range(KH):
                    ph = php.tile([P, tbs], f32, tag="ph")
                    mj, mo = divmod(m, HJ // P)
                    if isf8:
                        for k in range(KD // 2):
                            nc.tensor.matmul(
                                ph,
                                w1_sl[mj][:, 2 * k:2 * k + 2, mo * P:(mo + 1) * P],
                                xt_blk[:, 2 * k:2 * k + 2, :],
                                start=(k == 0),
                                stop=(k == KD // 2 - 1),
                                perf_mode=DR,
                            )
                        nc.scalar.activation(
                            hT[:, m, :], ph, Gelu,
                            bias=b1_sb[:, m:m + 1], scale=1.0 / (SX * SW1),
                        )
                    else:
                        for k in range(KD):
                            nc.tensor.matmul(
                                ph,
                                w1_sl[mj][:, k, mo * P:(mo + 1) * P],
                                xt_blk[:, k, :],
                                start=(k == 0),
                                stop=(k == KD - 1),
                            )
                        nc.scalar.activation(
                            hT[:, m, :], ph, Gelu, bias=b1_sb[:, m:m + 1]
                        )

                # --- mm2: yT[d] = (W2_chunk^T @ hT) * gate, one output DMA
                # per block (8 small stores per block cost ~0.6 us of SP
                # issue each and serialized the drain tail)
                yt_blk = ypool.tile([P, KD, tbs], fh, tag="yt")
                for d in range(KD):
                    pyT = pyp.tile([P, tbs], f32, tag="py")
                    if isf8:
                        for k in range(KH // 2):
                            nc.tensor.matmul(
                                pyT,
                                w28_sl[d][:, 2 * k:2 * k + 2, :],
                                hT[:, 2 * k:2 * k + 2, :],
                                start=(k == 0),
                                stop=(k == KH // 2 - 1),
                                perf_mode=DR,
                            )
                    else:
                        for k in range(KH):
                            nc.tensor.matmul(
                                pyT,
                                w2b_sl[d][:, k, :],
                                hT[:, k, :],
                                start=(k == 0),
                                stop=(k == KH - 1),
                            )
                    # fused PSUM evacuation + gate broadcast multiply on DVE
                    nc.vector.tensor_mul(yt_blk[:, d, :], pyT, gb_sb)
                    if b == nblk - 1:
                        # last block: store each d-slice as its own DMA so
                        # the drain pipelines across queues behind the PE
                        # instead of one serial descriptor chain at the end
                        nc.sync.dma_start(
                            ytr3[:, d, t0b:t0b + tbs], yt_blk[:, d, :]
                        )
                if b != nblk - 1:
                    nc.sync.dma_start(ytr3[:, :, t0b:t0b + tbs], yt_blk)
    nc.compile()
    return nc


def _ensure_trace_hooks():
    # bass_utils' trace path (taken when BASS_TRACE=1 is set externally)
    # imports antenv.axon_hooks, which this image lacks. Shim it (and the
    # artifact upload, which needs a bucket) only when missing, so tracing
    # degrades gracefully instead of crashing.
    import sys
    import types

    try:
        import antenv.axon_hooks  # noqa: F401
        return
    except ImportError:
        pass
    try:
        import antenv

        mod = types.ModuleType("antenv.axon_hooks")
        state = {"hook": None}
        mod.set_axon_ntff_profile_hook = lambda h: state.__setitem__("hook", h)
        mod.get_axon_ntff_profile_hook = lambda: state["hook"]
        sys.modules["antenv.axon_hooks"] = mod
        antenv.axon_hooks = mod
        try:
            from trn_agent_boot.trn_boot import _ntff_profile_via_ctypes

            mod.set_axon_ntff_profile_hook(
                _ntff_profile_via_ctypes("/opt/axon/libaxon_pjrt.so")
            )
            import concourse.bass_utils as _bu

            _orig_upload = _bu.upload_artifacts

            def _safe_upload(tmpdir):
                try:
                    return _orig_upload(tmpdir)
                except Exception:
                    return f"local:{tmpdir}"

            _bu.upload_artifacts = _safe_upload
        except Exception:
            pass
    except Exception:
        pass


def kernel(x, Wr, W1, b1, W2, b2):
    _ensure_trace_hooks()
    from concourse.bass_utils import run_bass_kernel_spmd

    f16 = np.float16
    e4m3 = ml_dtypes.float8_e4m3
    B, S, D = x.shape
    E, _, H = W1.shape
    N = B * S
    KD = D // P
    MJ = 16
    HJ = H // MJ
    xm = np.ascontiguousarray(x.reshape(N, D), dtype=np.float32)

    # --- host router (mirrors reference fp32 arithmetic; softmax is
    # monotonic so top-k on probs == top-k on logits, ties broken by index)
    logits = xm @ Wr
    mx = logits.max(axis=1, keepdims=True)
    ex = np.exp(logits - mx)
    probs = ex / ex.sum(axis=1, keepdims=True)
    top_i = np.argsort(-probs, axis=1, kind="stable")[:, :TOP_K]

    idx = [np.where((top_i == e).any(axis=1))[0] for e in range(E)]
    counts = np.array([len(i) for i in idx])
    cmax = int(counts.max())

    nbf = NBF_TARGET
    nf8 = max(NF8_TARGET, -((nbf - cmax) // 16) * 16)  # ceil16(cmax-nbf)
    sizes_a = _split_blocks(nbf, mult16=False)
    sizes_c = _split_blocks(nf8, mult16=True)
    ntot = nbf + nf8

    # --- dispatch: per expert, sort tokens by gate descending; largest nbf
    # gates -> tier A (bf16), rest -> tier C (fp8) + zero padding.
    xT = np.ascontiguousarray(xm.T).astype(f16)               # [D, N] fp16
    xT8 = np.ascontiguousarray((xm.T * SX)).astype(e4m3)      # [D, N] e4m3
    in_maps = []
    tok_a, tok_c = [], []
    for e in range(E):
        ge = probs[idx[e], e]
        order = np.argsort(-ge, kind="stable")
        ta = idx[e][order[:nbf]]
        tc = idx[e][order[nbf:]]
        tok_a.append(ta)
        tok_c.append(tc)

        def pack_blocks(src, toks, ncap, sizes, dt):
            # [P, KD*ncap] block-packed SBUF layout
            xte = np.zeros((D, ncap), dtype=dt)
            xte[:, :len(toks)] = src[:, toks]
            xte3 = xte.reshape(KD, P, ncap).transpose(1, 0, 2)
            t0 = 0
            chunks = []
            for tbs in sizes:
                chunks.append(xte3[:, :, t0:t0 + tbs].reshape(P, -1))
                t0 += tbs
            return np.ascontiguousarray(np.concatenate(chunks, axis=1))

        w1f = np.asarray(W1[e], dtype=np.float32)
        w2f = np.asarray(W2[e], dtype=np.float32)
        gfull = np.zeros((ntot,), dtype=np.float32)
        gfull[:len(ta)] = probs[ta, e]
        gfull[nbf:nbf + len(tc)] = probs[tc, e] / SW2
        in_maps.append({
            "xtb": pack_blocks(xT, ta, nbf, sizes_a, f16),
            "xt8": pack_blocks(xT8, tc, nf8, sizes_c, e4m3),
            "w1b": np.ascontiguousarray(
                w1f.astype(f16).reshape(KD, P, MJ, HJ).transpose(1, 2, 0, 3)
            ),
            "w18": np.ascontiguousarray(
                (w1f * SW1).astype(e4m3).reshape(KD, P, MJ, HJ).transpose(1, 2, 0, 3)
            ),
            "w2b": np.ascontiguousarray(
                w2f.astype(f16).reshape(H // P, P, KD, P).transpose(1, 2, 0, 3)
            ),
            "w28": np.ascontiguousarray(
                (w2f * SW2).astype(e4m3).reshape(H // P, P, KD, P).transpose(1, 2, 0, 3)
            ),
            "gb": np.ascontiguousarray(np.broadcast_to(gfull, (P, ntot)).astype(f16)),
            "b1t": np.ascontiguousarray(
                np.asarray(b1[e], dtype=np.float32).reshape(H // P, P).T
            ),
        })

    key = (tuple(sizes_a), tuple(sizes_c), D, H)
    if key not in _program_cache:
        _program_cache[key] = _build_program(sizes_a, sizes_c, D, H)
    nc = _program_cache[key]

    res = run_bass_kernel_spmd(nc, in_maps, core_ids=list(range(NUM_EXPERTS)))

    # --- combine: transpose each expert's [D, n] block and scatter-add
    # (indices unique per expert)
    out = np.zeros((N, D), dtype=np.float32)
    b2f = np.asarray(b2, dtype=np.float32)
    for e in range(E):
        ytr = np.asarray(res.results[e]["ytr"]).astype(np.float32)
        for toks, seg0 in ((tok_a[e], 0), (tok_c[e], nbf)):
            if not len(toks):
                continue
            ye = np.ascontiguousarray(ytr[:, seg0:seg0 + len(toks)].T)
            if b2f[e].any():
                ye = ye + probs[toks, e][:, None] * b2f[e]
            out[toks] += ye
    return out.reshape(B, S, D)


# revision 40
# speedup vs baseline: 1.0254x; 1.0206x over previous
# MoE (8 experts, top-2) on 8 TRN2 NeuronCores — expert-parallel, tiered
# precision.
#
# Host (numpy): router matmul + softmax + top-2 (mirrors the jax reference
# fp32 arithmetic), then per-expert dispatch into TWO tiers:
#   tier A (fp16):  the expert's nbf largest-gate tokens — fp16 matmuls
#                   (same PE rate as bf16, 8x finer mantissa: its error is
#                   ~0.9e-3 vs bf16's 3.4e-3, freeing budget for more fp8).
#   tier C (fp8):   the remaining tokens (smallest gates) + zero padding up
#                   to nf8 — e4m3 matmuls in DoubleRow perf mode (measured
#                   2.0x PE rate vs bf16 at FD>=300).
# (nbf, nf8) are uniform across cores so the single SPMD program fits every
# expert; capacity padding rides in the half-cost fp8 tier. Putting only
# low-gate tokens in fp8 keeps the quantization error a small fraction of
# the output norm (measured 1.83e-2 vs the 2e-2 gate; fp16-only is ~1e-3;
# the numpy e4m3 simulation in the dev harness predicts device error to
# ~1e-5, so the tier split was tuned against it).
#
# Device (per core, expert e), per token block:
#   mm1: hT = gelu(W1^T @ xT + b1)   (A: fp16, 8 K-chunks; C: e4m3 DoubleRow,
#        4 K-chunk-pairs, PSUM descaled 1/8192 inside the ACT op)
#   mm2: yT = (W2^T @ hT) * gate     (A: fp16; C: e4m3 DoubleRow with the
#        1/1024 W2 scale folded into the gate values — free)
# Both matmuls keep tokens on the moving/free axis so work scales with the
# exact token count. Gates and outputs are fp16 (halves the drain DMA).
#
# Weights are streamed per block (W1 slices, fp8 W2 d-slices) except fp16 W2
# which stays resident as 8 d-slices; SBUF peak ~26 MB, total DMA ~69
# MB/core — hidden under the ~390 us of PE time. Stream issue order is
# tuned so every tensor lands just before its first PE use (the in-order
# engine queues stall head-of-line otherwise).
#
# Shapes hardcoded for B=4, S=2048, D=1024, H=4096, E=8 (spec); the builder
# is parametric in the block lists (known only after routing).

import numpy as np
import ml_dtypes

NUM_EXPERTS = 8
TOP_K = 2
P = 128          # SBUF partitions
TB = 512         # token block (matmul moving free size)

# Tier sizes (tokens per expert): tuned offline on the seed-0 routing
# (counts 1932..2182, mean 2048). nbf + nf8 must cover the max expert load;
# the builder re-derives nf8 at runtime if routing ever exceeds it.
NBF_TARGET = 1344
NF8_TARGET = 848

# fp8 scales (powers of two; e4m3 max 240, scaled data max ~87)
SX = 16.0        # x
SW1 = 512.0      # W1  -> psum1 = 8192 * (x @ W1), descaled in the gelu ACT
SW2 = 1024.0     # W2  -> psum2 = 1024 * (h @ W2), folded into device gates

_program_cache = {}


def _split_blocks(n, mult16):
    """Split n tokens into blocks of <=TB, remainder last (block 0 must be
    full so its compute covers the next block's weight stream). For fp8
    (mult16) every block size must be a multiple of 16 (DoubleRow pair-slice
    stride) and at least 128 (DoubleRow is a net loss below FD=128), so a
    small remainder is rebalanced across the last two blocks."""
    if n == 0:
        return []
    sizes = [TB] * (n // TB)
    rem = n % TB
    if rem:
        sizes.append(rem)
    if mult16:
        assert n % 16 == 0
        if len(sizes) >= 2:
            # near-equal split: keeps every fp8 block >=256 (full DoubleRow
            # win) and shrinks the largest fp8 tile, which trims the h8/x8
            # SBUF footprint
            k = len(sizes)
            base = (n // k) // 16 * 16
            sizes = [base] * (k - 1) + [n - base * (k - 1)]
            assert sizes[-1] <= TB
    else:
        if len(sizes) >= 2 and sizes[-1] < 256:
            pair = sizes[-2] + sizes[-1]
            sizes[-2:] = [pair // 2, pair - pair // 2]
    return sizes


def _build_program(sizes_a, sizes_c, D, H):
    import concourse.mybir as mybir
    import concourse.tile as tile
    from concourse import bacc

    fh = mybir.dt.float16
    f8 = mybir.dt.float8e4
    f32 = mybir.dt.float32
    Gelu = mybir.ActivationFunctionType.Gelu_apprx_tanh
    DR = mybir.MatmulPerfMode.DoubleRow

    KD = D // P      # mm1 contraction chunks / mm2 output row chunks (8)
    KH = H // P      # mm1 output chunks / mm2 contraction chunks (32)
    MJ = 16          # W1 column-slice count (2 m-tiles per slice)
    HJ = H // MJ

    nbf = sum(sizes_a)
    nf8 = sum(sizes_c)
    ntot = nbf + nf8

    # Host-packed dram layouts (partition dim first, per-partition contiguous
    # runs >= 1KB so DMAs stream at full bandwidth):
    #   xtb : [P, KD*nbf] fp16    block-packed tier-A tokens
    #   xt8 : [P, KD*nf8] e4m3    block-packed tier-C tokens (x*16)
    #   w1b : [P, MJ, KD, HJ] fp16
    #   w18 : [P, MJ, KD, HJ] e4m3 (W1*512)
    #   w2b : [P, KD, KH, P]  fp16 (resident, d-major slices)
    #   w28 : [P, KD, KH, P]  e4m3 (W2*1024, d-major slices)
    #   gb  : [P, ntot] fp16      gates, tier-C segment pre-divided by 1024
    #   b1t : [P, KH] f32
    #   ytr : [D, ntot] fp16      transposed output, tier order A then C
    nc = bacc.Bacc(None, target_bir_lowering=False, debug=False)
    xtb = nc.declare_dram_parameter("xtb", [P, KD * max(nbf, 1)], fh, isOutput=False).ap()
    xt8 = nc.declare_dram_parameter("xt8", [P, KD * max(nf8, 1)], f8, isOutput=False).ap()
    w1b = nc.declare_dram_parameter("w1b", [P, MJ, KD, HJ], fh, isOutput=False).ap()
    w2b = nc.declare_dram_parameter("w2b", [P, KD, KH, P], fh, isOutput=False).ap()
    # per-C-position fp8 weights/bias: each C block position can be bound to
    # a DIFFERENT expert per core (the host bin-packs expert overflows into
    # uniform cells, cutting the fp8 capacity from ceil16(max overflow) to
    # near the mean overflow — worth ~10 us/core). Streaming volume is
    # unchanged: these were per-block streams already.
    w18P = [nc.declare_dram_parameter(f"w18_{p}", [P, MJ, KD, HJ], f8,
                                      isOutput=False).ap()
            for p in range(len(sizes_c))]
    w28P = [nc.declare_dram_parameter(f"w28_{p}", [P, KD, KH, P], f8,
                                      isOutput=False).ap()
            for p in range(len(sizes_c))]
    b1P = [nc.declare_dram_parameter(f"b1c_{p}", [P, KH], f32,
                                     isOutput=False).ap()
           for p in range(len(sizes_c))]
    gb = nc.declare_dram_parameter("gb", [P, ntot], fh, isOutput=False).ap()
    b1t = nc.declare_dram_parameter("b1t", [P, KH], f32, isOutput=False).ap()
    ytr = nc.declare_dram_parameter("ytr", [D, ntot], fh, isOutput=True).ap()
    # [P, KD, ntot] view of the [D, ntot] output for one-DMA-per-block stores
    ytr3 = ytr.rearrange("(d p) n -> p d n", p=P)

    # per-block metadata: (tbs, isf8, t0seg=offset within own tier, t0=global)
    blocks = []
    cpos_of = {}
    t0a = t0c = t0 = 0
    for tbs in sizes_a:
        blocks.append((tbs, False, t0a, t0))
        t0a += tbs
        t0 += tbs
    for p, tbs in enumerate(sizes_c):
        cpos_of[len(blocks)] = p
        blocks.append((tbs, True, t0c, t0))
        t0c += tbs
        t0 += tbs
    nblk = len(blocks)

    with tile.TileContext(nc) as tc:
        with (
            tc.tile_pool(name="wres", bufs=1) as wres,
            tc.tile_pool(name="wstr", bufs=4) as wstr,
            tc.tile_pool(name="data", bufs=2) as datap,
            tc.tile_pool(name="psum", bufs=4, space="PSUM") as psump,
        ):
            # merged pools (fewer pools -> shorter entry/exit barrier
            # handshake); rotation depth is set per tag via bufs=
            w1sp = w28p = wstr
            xpool = hpool = ypool = gbp = datap
            php = pyp = psump
            b1_sb = wres.tile([P, KH], f32, tag="b1sb")
            b1c_sb = [
                wres.tile([P, KH], f32, tag=f"b1c{p}", name=f"b1c{p}")
                for p in range(len(sizes_c))
            ]
            # resident bf16 W2 as 8 d-major slices: the tile scheduler
            # hoists a few mm2 matmuls high into the in-order PE queue, so
            # the first d-slice must land early (a monolithic 8.4 MB DMA
            # stalled the PE ~10-15 us).
            w2b_sl = [
                wres.tile([P, KH, P], fh, tag=f"w2bd{d}", name=f"w2bd{d}")
                for d in range(KD)
            ]

            # per-block input tiles, possibly issued one block ahead
            xt_tiles = {}
            gb_tiles = {}
            w1_tiles = {b: [None] * MJ for b in range(nblk)}

            def issue_x(b):
                tbs, isf8, t0seg, t0b = blocks[b]
                if isf8:
                    xt_blk = xpool.tile([P, KD, tbs], f8, tag="x8")
                    src, base = xt8, KD * t0seg
                else:
                    xt_blk = xpool.tile([P, KD, tbs], fh, tag="xt")
                    src, base = xtb, KD * t0seg
                if b == 0:
                    # chunk 0 alone first (the very first matmul's only x
                    # dependency — a 128 KB transfer that lands fast), then
                    # the rest split across queues
                    nc.sync.dma_start(
                        xt_blk[:, 0, :], src[:, base:base + tbs]
                    )
                    for k in range(1, KD):
                        nc.sync.dma_start(
                            xt_blk[:, k, :],
                            src[:, base + k * tbs:base + (k + 1) * tbs],
                        )
                else:
                    nc.sync.dma_start(
                        xt_blk,
                        src[:, base:base + KD * tbs].rearrange(
                            "p (k c) -> p k c", k=KD
                        ),
                    )
                xt_tiles[b] = xt_blk
                gb_sb = gbp.tile([P, tbs], fh, tag="gb")
                nc.sync.dma_start(gb_sb, gb[:, t0b:t0b + tbs])
                gb_tiles[b] = gb_sb

            def issue_w1(b, js):
                _, isf8, _, _ = blocks[b]
                wsrc, wdt, wtag = ((w18P[cpos_of[b]], f8, "w18s") if isf8
                                   else (w1b, fh, "w1bs"))
                for j in js:
                    if w1_tiles[b][j] is not None:
                        continue
                    ws = w1sp.tile([P, KD, HJ], wdt, tag=wtag)
                    nc.sync.dma_start(ws, wsrc[:, j, :, :])
                    w1_tiles[b][j] = ws

            # PE pstate warmup: the engine runs its first ~3 us at a reduced
            # p-state (first real m-tile measured 427-634 ns/matmul instead
            # of 216). Zero matmuls on memset tiles ramp it up during the
            # startup DMA window; they retire before the first x/W1 slice
            # lands (~11 us), so they never delay real work.
            warm_l = wres.tile([P, P], fh, tag="warm_l")
            warm_r = wres.tile([P, TB // 2], fh, tag="warm_r")
            nc.vector.memset(warm_l, 0.0)
            nc.vector.memset(warm_r, 0.0)
            for _ in range(24):
                pw = php.tile([P, TB // 2], f32, tag="ph")
                nc.tensor.matmul(pw, warm_l, warm_r, start=True, stop=True)

            issue_w1(0, range(0, 1))
            issue_x(0)
            nc.sync.dma_start(b1_sb, b1t)
            for p in range(len(sizes_c)):
                nc.sync.dma_start(b1c_sb[p], b1P[p])
            issue_w1(0, range(1, 4))
            # d0 early: the scheduler hoists a few mm2-d0 matmuls between
            # mm1 m-tiles ~6-7, so d0 must land by ~25 us. d1..d7 are only
            # needed at the real mm2 (~105 us) — after the W1 stream.
            nc.sync.dma_start(w2b_sl[0], w2b[:, 0, :, :])
            issue_w1(0, range(4, MJ))
            for d in range(1, KD):
                nc.sync.dma_start(w2b_sl[d], w2b[:, d, :, :])

            for b, (tbs, isf8, t0seg, t0b) in enumerate(blocks):
                # finish this block's input streams (x/gb/first W1 slices
                # were issued during the previous block). For fp8 blocks the
                # W2 d-slices are interleaved into the W1 stream by need
                # time (W1 slice j feeds m-tiles 2j at ~j*1.05 us; W2 slice
                # d feeds mm2 at ~mm1_end + d*2.6 us) — issuing all W1
                # first made mm2's first d-slices arrive late (~3.5 us PE
                # stall per fp8 block).
                w28_sl = []
                if isf8:
                    w28src = w28P[cpos_of[b]]

                    def issue_w28(dds):
                        for dd in dds:
                            w2s = w28p.tile([P, KH, P], f8, tag="w28s", bufs=8)
                            nc.sync.dma_start(w2s, w28src[:, dd, :, :])
                            w28_sl.append(w2s)
                    issue_w1(b, range(0, 10))
                    issue_w28(range(0, 2))
                    issue_w1(b, range(10, MJ))
                    issue_w28(range(2, KD))
                else:
                    issue_w1(b, range(MJ))
                # prefetch the next block's x, gates, and first W1 slices so
                # the A->C phase switch doesn't wait behind this block's
                # 8.4 MB W1 stream
                if b + 1 < nblk:
                    issue_x(b + 1)
                    issue_w1(b + 1, range(0, 3))

                xt_blk = xt_tiles.pop(b)
                gb_sb = gb_tiles.pop(b)
                w1_sl = w1_tiles.pop(b)

                # --- mm1: hT[m] = gelu(W1_chunk^T @ xT + b1) -> [P, tbs]
                hT = hpool.tile([P, KH, tbs], f8 if isf8 else fh,
                                tag="h8" if isf8 else "hT", bufs=1)
                for m in range(KH):
                    ph = php.tile([P, tbs], f32, tag="ph")
                    mj, mo = divmod(m, HJ // P)
                    if isf8:
                        for k in range(KD // 2):
                            nc.tensor.matmul(
                                ph,
                                w1_sl[mj][:, 2 * k:2 * k + 2, mo * P:(mo + 1) * P],
                                xt_blk[:, 2 * k:2 * k + 2, :],
                                start=(k == 0),
                                stop=(k == KD // 2 - 1),
                                perf_mode=DR,
                            )
                        nc.scalar.activation(
                            hT[:, m, :], ph, Gelu,
                            bias=b1c_sb[cpos_of[b]][:, m:m + 1],
                            scale=1.0 / (SX * SW1),
                        )
                    else:
                        for k in range(KD):
                            nc.tensor.matmul(
                                ph,
                                w1_sl[mj][:, k, mo * P:(mo + 1) * P],
                                xt_blk[:, k, :],
                                start=(k == 0),
                                stop=(k == KD - 1),
                            )
                        nc.scalar.activation(
                            hT[:, m, :], ph, Gelu, bias=b1_sb[:, m:m + 1]
                        )

                # --- mm2: yT[d] = (W2_chunk^T @ hT) * gate, one output DMA
                # per block (8 small stores per block cost ~0.6 us of SP
                # issue each and serialized the drain tail)
                yt_blk = ypool.tile([P, KD, tbs], fh, tag="yt")
                for d in range(KD):
                    pyT = pyp.tile([P, tbs], f32, tag="py")
                    if isf8:
                        for k in range(KH // 2):
                            nc.tensor.matmul(
                                pyT,
                                w28_sl[d][:, 2 * k:2 * k + 2, :],
                                hT[:, 2 * k:2 * k + 2, :],
                                start=(k == 0),
                                stop=(k == KH // 2 - 1),
                                perf_mode=DR,
                            )
                    else:
                        for k in range(KH):
                            nc.tensor.matmul(
                                pyT,
                                w2b_sl[d][:, k, :],
                                hT[:, k, :],
                                start=(k == 0),
                                stop=(k == KH - 1),
                            )
                    # fused PSUM evacuation + gate broadcast multiply on DVE
                    nc.vector.tensor_mul(yt_blk[:, d, :], pyT, gb_sb)
                    if b == nblk - 1:
                        # last block: store each d-slice as its own DMA so
                        # the drain pipelines across queues behind the PE
                        # instead of one serial descriptor chain at the end
                        nc.sync.dma_start(
                            ytr3[:, d, t0b:t0b + tbs], yt_blk[:, d, :]
                        )
                if b != nblk - 1:
                    nc.sync.dma_start(ytr3[:, :, t0b:t0b + tbs], yt_blk)
    nc.compile()
    return nc


def _ensure_trace_hooks():
    # bass_utils' trace path (taken when BASS_TRACE=1 is set externally)
    # imports antenv.axon_hooks, which this image lacks. Shim it (and the
    # artifact upload, which needs a bucket) only when missing, so tracing
    # degrades gracefully instead of crashing.
    import sys
    import types

    try:
        import antenv.axon_hooks  # noqa: F401
        return
    except ImportError:
        pass
    try:
        import antenv

        mod = types.ModuleType("antenv.axon_hooks")
        state = {"hook": None}
        mod.set_axon_ntff_profile_hook = lambda h: state.__setitem__("hook", h)
        mod.get_axon_ntff_profile_hook = lambda: state["hook"]
        sys.modules["antenv.axon_hooks"] = mod
        antenv.axon_hooks = mod
        try:
            from trn_agent_boot.trn_boot import _ntff_profile_via_ctypes

            mod.set_axon_ntff_profile_hook(
                _ntff_profile_via_ctypes("/opt/axon/libaxon_pjrt.so")
            )
            import concourse.bass_utils as _bu

            _orig_upload = _bu.upload_artifacts

            def _safe_upload(tmpdir):
                try:
                    return _orig_upload(tmpdir)
                except Exception:
                    return f"local:{tmpdir}"

            _bu.upload_artifacts = _safe_upload
        except Exception:
            pass
    except Exception:
        pass


def _plan_cells(fe):
    """Bin-pack per-expert fp8 overflows fe into 8 uniform (s0, s1) cell
    pairs (one pair per core), single-expert cells, cells splittable across
    cores. Returns (sizes_c, assign) with assign[k][p] = (expert, lo, hi)
    meaning core k's C position p holds tokens [lo:hi) of that expert's fp8
    token list (None cell = all padding). Falls back to own-expert cells
    sized by the max overflow if no candidate packs."""
    import itertools
    E = len(fe)
    for sizes in [(432, 320), (448, 320), (448, 336), (464, 352),
                  (480, 368), (496, 384), (512, 416), (512, 512)]:
        s0, s1 = sizes
        # per-expert Pareto-minimal (a, b) cell counts
        opts = []
        for d in fe:
            o = [(a, b) for a in range(4) for b in range(4)
                 if a * s0 + b * s1 >= d]
            o = [c for c in o if not any(
                c2 != c and c2[0] <= c[0] and c2[1] <= c[1] for c2 in o)]
            opts.append(o)
        # DP with parent pointers over cumulative (na, nb) <= (8, 8)
        states = {(0, 0): []}
        ok = True
        for o in opts:
            nxt = {}
            for (na, nb), path in states.items():
                for a, b in o:
                    ns = (na + a, nb + b)
                    if ns[0] <= 8 and ns[1] <= 8 and ns not in nxt:
                        nxt[ns] = path + [(a, b)]
            if not nxt:
                ok = False
                break
            states = nxt
        if not ok:
            continue
        counts_ab = next(iter(states.values()))
        # materialize cells: position-0 cells then position-1 cells, each a
        # (expert, lo, hi) chunk of the expert's fp8 token list
        cells0, cells1 = [], []
        for e, (a, b) in enumerate(counts_ab):
            off = 0
            for _ in range(a):
                take = min(s0, max(0, fe[e] - off))
                cells0.append((e, off, off + take))
                off += take
            for _ in range(b):
                take = min(s1, max(0, fe[e] - off))
                cells1.append((e, off, off + take))
                off += take
        cells0 += [None] * (8 - len(cells0))
        cells1 += [None] * (8 - len(cells1))
        assign = [[cells0[k], cells1[k]] for k in range(8)]
        return [s0, s1], assign
    # fallback: own-expert cells, capacity = ceil16(max overflow)
    nf8 = max(256, -((-max(fe)) // 16) * 16)
    sizes_c = _split_blocks(nf8, mult16=True)
    assign = []
    for k in range(8):
        row, off = [], 0
        for tbs in sizes_c:
            take = min(tbs, max(0, fe[k] - off))
            row.append((k, off, off + take))
            off += take
        assign.append(row)
    return sizes_c, assign


def kernel(x, Wr, W1, b1, W2, b2):
    _ensure_trace_hooks()
    from concourse.bass_utils import run_bass_kernel_spmd

    f16 = np.float16
    e4m3 = ml_dtypes.float8_e4m3
    B, S, D = x.shape
    E, _, H = W1.shape
    N = B * S
    KD = D // P
    MJ = 16
    HJ = H // MJ
    xm = np.ascontiguousarray(x.reshape(N, D), dtype=np.float32)

    # --- host router (mirrors reference fp32 arithmetic; softmax is
    # monotonic so top-k on probs == top-k on logits, ties broken by index)
    logits = xm @ Wr
    mx = logits.max(axis=1, keepdims=True)
    ex = np.exp(logits - mx)
    probs = ex / ex.sum(axis=1, keepdims=True)
    top_i = np.argsort(-probs, axis=1, kind="stable")[:, :TOP_K]

    idx = [np.where((top_i == e).any(axis=1))[0] for e in range(E)]
    counts = np.array([len(i) for i in idx])
    cmax = int(counts.max())

    nbf = NBF_TARGET
    fe = [max(0, int(counts[e]) - nbf) for e in range(E)]
    sizes_a = _split_blocks(nbf, mult16=False)
    sizes_c, assign = _plan_cells(fe)
    nf8 = sum(sizes_c)
    ntot = nbf + nf8

    # --- dispatch: per expert, sort tokens by gate descending; largest nbf
    # gates -> tier A (bf16), rest -> tier C (fp8) + zero padding.
    xT = np.ascontiguousarray(xm.T).astype(f16)               # [D, N] fp16
    xT8 = np.ascontiguousarray((xm.T * SX)).astype(e4m3)      # [D, N] e4m3

    def pack_blocks(src, toks, ncap, sizes, dt):
        # [P, KD*ncap] block-packed SBUF layout
        xte = np.zeros((D, ncap), dtype=dt)
        xte[:, :len(toks)] = src[:, toks]
        xte3 = xte.reshape(KD, P, ncap).transpose(1, 0, 2)
        t0 = 0
        chunks = []
        for tbs in sizes:
            chunks.append(xte3[:, :, t0:t0 + tbs].reshape(P, -1))
            t0 += tbs
        return np.ascontiguousarray(np.concatenate(chunks, axis=1))

    # per-expert packed tensors (referenced by whichever core uses them)
    tok_a, tok_c = [], []
    w1bp, w18p_, w2bp, w28p_, b1p_ = [], [], [], [], []
    for e in range(E):
        ge = probs[idx[e], e]
        order = np.argsort(-ge, kind="stable")
        tok_a.append(idx[e][order[:nbf]])
        tok_c.append(idx[e][order[nbf:]])
        w1f = np.asarray(W1[e], dtype=np.float32)
        w2f = np.asarray(W2[e], dtype=np.float32)
        w1bp.append(np.ascontiguousarray(
            w1f.astype(f16).reshape(KD, P, MJ, HJ).transpose(1, 2, 0, 3)))
        w18p_.append(np.ascontiguousarray(
            (w1f * SW1).astype(e4m3).reshape(KD, P, MJ, HJ).transpose(1, 2, 0, 3)))
        w2bp.append(np.ascontiguousarray(
            w2f.astype(f16).reshape(H // P, P, KD, P).transpose(1, 2, 0, 3)))
        w28p_.append(np.ascontiguousarray(
            (w2f * SW2).astype(e4m3).reshape(H // P, P, KD, P).transpose(1, 2, 0, 3)))
        b1p_.append(np.ascontiguousarray(
            np.asarray(b1[e], dtype=np.float32).reshape(H // P, P).T))

    in_maps = []
    for k in range(E):
        ta = tok_a[k]
        # core k's C tokens: concatenation of its assigned cells' slices,
        # each padded to the cell size
        ctoks = []      # per cell: token array (len <= cell size)
        for p, cell in enumerate(assign[k]):
            if cell is None:
                ctoks.append(np.zeros(0, dtype=np.int64))
            else:
                e, lo, hi = cell
                ctoks.append(tok_c[e][lo:hi])
        # x dispatch: pack each cell into its block slot
        xc = np.zeros((D, ntot - nbf), dtype=e4m3)
        gfull = np.zeros((ntot,), dtype=np.float32)
        gfull[:len(ta)] = probs[ta, k]
        off = 0
        for p, tbs in enumerate(sizes_c):
            t = ctoks[p]
            if len(t):
                xc[:, off:off + len(t)] = xT8[:, t]
                e = assign[k][p][0]
                gfull[nbf + off:nbf + off + len(t)] = probs[t, e] / SW2
            off += tbs
        im = {
            "xtb": pack_blocks(xT, ta, nbf, sizes_a, f16),
            "xt8": pack_blocks(xc, np.arange(ntot - nbf), ntot - nbf,
                               sizes_c, e4m3),
            "w1b": w1bp[k],
            "w2b": w2bp[k],
            "gb": np.ascontiguousarray(
                np.broadcast_to(gfull, (P, ntot)).astype(f16)),
            "b1t": b1p_[k],
        }
        for p in range(len(sizes_c)):
            e = assign[k][p][0] if assign[k][p] is not None else 0
            im[f"w18_{p}"] = w18p_[e]
            im[f"w28_{p}"] = w28p_[e]
            im[f"b1c_{p}"] = b1p_[e]
        in_maps.append(im)

    key = (tuple(sizes_a), tuple(sizes_c), D, H)
    if key not in _program_cache:
        _program_cache[key] = _build_program(sizes_a, sizes_c, D, H)
    nc = _program_cache[key]

    res = run_bass_kernel_spmd(nc, in_maps, core_ids=list(range(NUM_EXPERTS)))

    # --- combine: per core, the A segment holds its own expert's tokens,
    # each C cell holds its assigned expert's token slice
    out = np.zeros((N, D), dtype=np.float32)
    b2f = np.asarray(b2, dtype=np.float32)
    for k in range(E):
        ytr = np.asarray(res.results[k]["ytr"]).astype(np.float32)
        segs = [(tok_a[k], k, 0)]
        off = nbf
        for p, tbs in enumerate(sizes_c):
            cell = assign[k][p]
            if cell is not None:
                e, lo, hi = cell
                segs.append((tok_c[e][lo:hi], e, off))
            off += tbs
        for toks, e, seg0 in segs:
            if not len(toks):
                continue
            ye = np.ascontiguousarray(ytr[:, seg0:seg0 + len(toks)].T)
            if b2f[e].any():
                ye = ye + probs[toks, e][:, None] * b2f[e]
            out[toks] += ye
    return out.reshape(B, S, D)


# revision 42
# speedup vs baseline: 1.0293x; 1.0039x over previous
# MoE (8 experts, top-2) on 8 TRN2 NeuronCores — expert-parallel, tiered
# precision.
#
# Host (numpy): router matmul + softmax + top-2 (mirrors the jax reference
# fp32 arithmetic), then per-expert dispatch into TWO tiers:
#   tier A (fp16):  the expert's nbf largest-gate tokens — fp16 matmuls
#                   (same PE rate as bf16, 8x finer mantissa: its error is
#                   ~0.9e-3 vs bf16's 3.4e-3, freeing budget for more fp8).
#   tier C (fp8):   the remaining tokens (smallest gates) + zero padding up
#                   to nf8 — e4m3 matmuls in DoubleRow perf mode (measured
#                   2.0x PE rate vs bf16 at FD>=300).
# (nbf, nf8) are uniform across cores so the single SPMD program fits every
# expert; capacity padding rides in the half-cost fp8 tier. Putting only
# low-gate tokens in fp8 keeps the quantization error a small fraction of
# the output norm (measured 1.83e-2 vs the 2e-2 gate; fp16-only is ~1e-3;
# the numpy e4m3 simulation in the dev harness predicts device error to
# ~1e-5, so the tier split was tuned against it).
#
# Device (per core, expert e), per token block:
#   mm1: hT = gelu(W1^T @ xT + b1)   (A: fp16, 8 K-chunks; C: e4m3 DoubleRow,
#        4 K-chunk-pairs, PSUM descaled 1/8192 inside the ACT op)
#   mm2: yT = (W2^T @ hT) * gate     (A: fp16; C: e4m3 DoubleRow with the
#        1/1024 W2 scale folded into the gate values — free)
# Both matmuls keep tokens on the moving/free axis so work scales with the
# exact token count. Gates and outputs are fp16 (halves the drain DMA).
#
# Weights are streamed per block (W1 slices, fp8 W2 d-slices) except fp16 W2
# which stays resident as 8 d-slices; SBUF peak ~26 MB, total DMA ~69
# MB/core — hidden under the ~390 us of PE time. Stream issue order is
# tuned so every tensor lands just before its first PE use (the in-order
# engine queues stall head-of-line otherwise).
#
# Shapes hardcoded for B=4, S=2048, D=1024, H=4096, E=8 (spec); the builder
# is parametric in the block lists (known only after routing).

import numpy as np
import ml_dtypes

NUM_EXPERTS = 8
TOP_K = 2
P = 128          # SBUF partitions
TB = 512         # token block (matmul moving free size)

# Tier sizes (tokens per expert): tuned offline on the seed-0 routing
# (counts 1932..2182, mean 2048). nbf + nf8 must cover the max expert load;
# the builder re-derives nf8 at runtime if routing ever exceeds it.
NBF_TARGET = 1344
NF8_TARGET = 848

# fp8 scales (powers of two; e4m3 max 240, scaled data max ~87)
SX = 16.0        # x
SW1 = 512.0      # W1  -> psum1 = 8192 * (x @ W1), descaled in the gelu ACT
SW2 = 1024.0     # W2  -> psum2 = 1024 * (h @ W2), folded into device gates

_program_cache = {}


def _split_blocks(n, mult16):
    """Split n tokens into blocks of <=TB, remainder last (block 0 must be
    full so its compute covers the next block's weight stream). For fp8
    (mult16) every block size must be a multiple of 16 (DoubleRow pair-slice
    stride) and at least 128 (DoubleRow is a net loss below FD=128), so a
    small remainder is rebalanced across the last two blocks."""
    if n == 0:
        return []
    sizes = [TB] * (n // TB)
    rem = n % TB
    if rem:
        sizes.append(rem)
    if mult16:
        assert n % 16 == 0
        if len(sizes) >= 2:
            # near-equal split: keeps every fp8 block >=256 (full DoubleRow
            # win) and shrinks the largest fp8 tile, which trims the h8/x8
            # SBUF footprint
            k = len(sizes)
            base = (n // k) // 16 * 16
            sizes = [base] * (k - 1) + [n - base * (k - 1)]
            assert sizes[-1] <= TB
    else:
        if len(sizes) >= 2 and sizes[-1] < 256:
            pair = sizes[-2] + sizes[-1]
            sizes[-2:] = [pair // 2, pair - pair // 2]
    return sizes


def _build_program(sizes_a, sizes_c, D, H):
    import concourse.mybir as mybir
    import concourse.tile as tile
    from concourse import bacc

    fh = mybir.dt.float16
    f8 = mybir.dt.float8e4
    f32 = mybir.dt.float32
    Gelu = mybir.ActivationFunctionType.Gelu_apprx_tanh
    DR = mybir.MatmulPerfMode.DoubleRow

    KD = D // P      # mm1 contraction chunks / mm2 output row chunks (8)
    KH = H // P      # mm1 output chunks / mm2 contraction chunks (32)
    MJ = 16          # W1 column-slice count (2 m-tiles per slice)
    HJ = H // MJ

    nbf = sum(sizes_a)
    nf8 = sum(sizes_c)
    ntot = nbf + nf8

    # Host-packed dram layouts (partition dim first, per-partition contiguous
    # runs >= 1KB so DMAs stream at full bandwidth):
    #   xtb : [P, KD*nbf] fp16    block-packed tier-A tokens
    #   xt8 : [P, KD*nf8] e4m3    block-packed tier-C tokens (x*16)
    #   w1b : [P, MJ, KD, HJ] fp16
    #   w18 : [P, MJ, KD, HJ] e4m3 (W1*512)
    #   w2b : [P, KD, KH, P]  fp16 (resident, d-major slices)
    #   w28 : [P, KD, KH, P]  e4m3 (W2*1024, d-major slices)
    #   gb  : [P, ntot] fp16      gates, tier-C segment pre-divided by 1024
    #   b1t : [P, KH] f32
    #   ytr : [D, ntot] fp16      transposed output, tier order A then C
    nc = bacc.Bacc(None, target_bir_lowering=False, debug=False)
    xtb = nc.declare_dram_parameter("xtb", [P, KD * max(nbf, 1)], fh, isOutput=False).ap()
    xt8 = nc.declare_dram_parameter("xt8", [P, KD * max(nf8, 1)], f8, isOutput=False).ap()
    w1b = nc.declare_dram_parameter("w1b", [P, MJ, KD, HJ], fh, isOutput=False).ap()
    w2b = nc.declare_dram_parameter("w2b", [P, KD, KH, P], fh, isOutput=False).ap()
    # per-C-position fp8 weights/bias: each C block position can be bound to
    # a DIFFERENT expert per core (the host bin-packs expert overflows into
    # uniform cells, cutting the fp8 capacity from ceil16(max overflow) to
    # near the mean overflow — worth ~10 us/core). Streaming volume is
    # unchanged: these were per-block streams already.
    w18P = [nc.declare_dram_parameter(f"w18_{p}", [P, MJ, KD, HJ], f8,
                                      isOutput=False).ap()
            for p in range(len(sizes_c))]
    w28P = [nc.declare_dram_parameter(f"w28_{p}", [P, KD, KH, P], f8,
                                      isOutput=False).ap()
            for p in range(len(sizes_c))]
    b1P = [nc.declare_dram_parameter(f"b1c_{p}", [P, KH], f32,
                                     isOutput=False).ap()
           for p in range(len(sizes_c))]
    gb = nc.declare_dram_parameter("gb", [P, ntot], fh, isOutput=False).ap()
    b1t = nc.declare_dram_parameter("b1t", [P, KH], f32, isOutput=False).ap()
    ytr = nc.declare_dram_parameter("ytr", [D, ntot], fh, isOutput=True).ap()
    # [P, KD, ntot] view of the [D, ntot] output for one-DMA-per-block stores
    ytr3 = ytr.rearrange("(d p) n -> p d n", p=P)

    # per-block metadata: (tbs, isf8, t0seg=offset within own tier, t0=global)
    blocks = []
    cpos_of = {}
    t0a = t0c = t0 = 0
    for tbs in sizes_a:
        blocks.append((tbs, False, t0a, t0))
        t0a += tbs
        t0 += tbs
    for p, tbs in enumerate(sizes_c):
        cpos_of[len(blocks)] = p
        blocks.append((tbs, True, t0c, t0))
        t0c += tbs
        t0 += tbs
    nblk = len(blocks)

    with tile.TileContext(nc) as tc:
        with (
            tc.tile_pool(name="wres", bufs=1) as wres,
            tc.tile_pool(name="wstr", bufs=4) as wstr,
            tc.tile_pool(name="data", bufs=2) as datap,
            tc.tile_pool(name="psum", bufs=4, space="PSUM") as psump,
        ):
            # merged pools (fewer pools -> shorter entry/exit barrier
            # handshake); rotation depth is set per tag via bufs=
            w1sp = w28p = wstr
            xpool = hpool = ypool = gbp = datap
            php = pyp = psump
            b1_sb = wres.tile([P, KH], f32, tag="b1sb")
            b1c_sb = [
                wres.tile([P, KH], f32, tag=f"b1c{p}", name=f"b1c{p}")
                for p in range(len(sizes_c))
            ]
            # resident bf16 W2 as 8 d-major slices: the tile scheduler
            # hoists a few mm2 matmuls high into the in-order PE queue, so
            # the first d-slice must land early (a monolithic 8.4 MB DMA
            # stalled the PE ~10-15 us).
            w2b_sl = [
                wres.tile([P, KH, P], fh, tag=f"w2bd{d}", name=f"w2bd{d}")
                for d in range(KD)
            ]

            # per-block input tiles, possibly issued one block ahead
            xt_tiles = {}
            gb_tiles = {}
            w1_tiles = {b: [None] * MJ for b in range(nblk)}

            def issue_x(b):
                tbs, isf8, t0seg, t0b = blocks[b]
                if isf8:
                    xt_blk = xpool.tile([P, KD, tbs], f8, tag="x8")
                    src, base = xt8, KD * t0seg
                else:
                    xt_blk = xpool.tile([P, KD, tbs], fh, tag="xt")
                    src, base = xtb, KD * t0seg
                if b == 0:
                    # block-0 x chunks issued from the DVE ring: SP's
                    # descriptor-issue rate (~0.6 us per DMACopy) serializes
                    # the ~12 startup descriptors otherwise, delaying the
                    # first matmul's data by several us
                    for k in range(KD):
                        nc.scalar.dma_start(
                            xt_blk[:, k, :],
                            src[:, base + k * tbs:base + (k + 1) * tbs],
                        )
                else:
                    nc.sync.dma_start(
                        xt_blk,
                        src[:, base:base + KD * tbs].rearrange(
                            "p (k c) -> p k c", k=KD
                        ),
                    )
                xt_tiles[b] = xt_blk
                gb_sb = gbp.tile([P, tbs], fh, tag="gb")
                nc.sync.dma_start(gb_sb, gb[:, t0b:t0b + tbs])
                gb_tiles[b] = gb_sb

            def issue_w1(b, js):
                _, isf8, _, _ = blocks[b]
                wsrc, wdt, wtag = ((w18P[cpos_of[b]], f8, "w18s") if isf8
                                   else (w1b, fh, "w1bs"))
                for j in js:
                    if w1_tiles[b][j] is not None:
                        continue
                    ws = w1sp.tile([P, KD, HJ], wdt, tag=wtag)
                    nc.sync.dma_start(ws, wsrc[:, j, :, :])
                    w1_tiles[b][j] = ws

            # PE pstate warmup: the engine runs its first ~3 us at a reduced
            # p-state (first real m-tile measured 427-634 ns/matmul instead
            # of 216). Zero matmuls on memset tiles ramp it up during the
            # startup DMA window; they retire before the first x/W1 slice
            # lands (~11 us), so they never delay real work.
            warm_l = wres.tile([P, P], fh, tag="warm_l")
            warm_r = wres.tile([P, TB // 2], fh, tag="warm_r")
            nc.vector.memset(warm_l, 0.0)
            nc.vector.memset(warm_r, 0.0)
            for _ in range(24):
                pw = php.tile([P, TB // 2], f32, tag="ph")
                nc.tensor.matmul(pw, warm_l, warm_r, start=True, stop=True)

            issue_w1(0, range(0, 1))
            issue_x(0)
            nc.gpsimd.dma_start(b1_sb, b1t)
            for p in range(len(sizes_c)):
                nc.gpsimd.dma_start(b1c_sb[p], b1P[p])
            issue_w1(0, range(1, 4))
            # d0 early: the scheduler hoists a few mm2-d0 matmuls between
            # mm1 m-tiles ~6-7, so d0 must land by ~25 us. d1..d7 are only
            # needed at the real mm2 (~105 us) — after the W1 stream.
            nc.sync.dma_start(w2b_sl[0], w2b[:, 0, :, :])
            issue_w1(0, range(4, MJ))
            for d in range(1, KD):
                nc.sync.dma_start(w2b_sl[d], w2b[:, d, :, :])

            for b, (tbs, isf8, t0seg, t0b) in enumerate(blocks):
                # finish this block's input streams (x/gb/first W1 slices
                # were issued during the previous block). For fp8 blocks the
                # W2 d-slices are interleaved into the W1 stream by need
                # time (W1 slice j feeds m-tiles 2j at ~j*1.05 us; W2 slice
                # d feeds mm2 at ~mm1_end + d*2.6 us) — issuing all W1
                # first made mm2's first d-slices arrive late (~3.5 us PE
                # stall per fp8 block).
                w28_sl = []
                if isf8:
                    w28src = w28P[cpos_of[b]]

                    def issue_w28(dds):
                        for dd in dds:
                            w2s = w28p.tile([P, KH, P], f8, tag="w28s", bufs=8)
                            nc.sync.dma_start(w2s, w28src[:, dd, :, :])
                            w28_sl.append(w2s)
                    issue_w1(b, range(0, 10))
                    issue_w28(range(0, 2))
                    issue_w1(b, range(10, MJ))
                    issue_w28(range(2, KD))
                else:
                    issue_w1(b, range(MJ))
                # prefetch the next block's x, gates, and first W1 slices so
                # the A->C phase switch doesn't wait behind this block's
                # 8.4 MB W1 stream
                if b + 1 < nblk:
                    issue_x(b + 1)
                    issue_w1(b + 1, range(0, 3))

                xt_blk = xt_tiles.pop(b)
                gb_sb = gb_tiles.pop(b)
                w1_sl = w1_tiles.pop(b)

                # --- mm1: hT[m] = gelu(W1_chunk^T @ xT + b1) -> [P, tbs]
                hT = hpool.tile([P, KH, tbs], f8 if isf8 else fh,
                                tag="h8" if isf8 else "hT", bufs=1)
                for m in range(KH):
                    ph = php.tile([P, tbs], f32, tag="ph")
                    mj, mo = divmod(m, HJ // P)
                    if isf8:
                        for k in range(KD // 2):
                            nc.tensor.matmul(
                                ph,
                                w1_sl[mj][:, 2 * k:2 * k + 2, mo * P:(mo + 1) * P],
                                xt_blk[:, 2 * k:2 * k + 2, :],
                                start=(k == 0),
                                stop=(k == KD // 2 - 1),
                                perf_mode=DR,
                            )
                        nc.scalar.activation(
                            hT[:, m, :], ph, Gelu,
                            bias=b1c_sb[cpos_of[b]][:, m:m + 1],
                            scale=1.0 / (SX * SW1),
                        )
                    else:
                        for k in range(KD):
                            nc.tensor.matmul(
                                ph,
                                w1_sl[mj][:, k, mo * P:(mo + 1) * P],
                                xt_blk[:, k, :],
                                start=(k == 0),
                                stop=(k == KD - 1),
                            )
                        nc.scalar.activation(
                            hT[:, m, :], ph, Gelu, bias=b1_sb[:, m:m + 1]
                        )

                # --- mm2: yT[d] = (W2_chunk^T @ hT) * gate, one output DMA
                # per block (8 small stores per block cost ~0.6 us of SP
                # issue each and serialized the drain tail)
                yt_blk = ypool.tile([P, KD, tbs], fh, tag="yt")
                for d in range(KD):
                    pyT = pyp.tile([P, tbs], f32, tag="py")
                    if isf8:
                        for k in range(KH // 2):
                            nc.tensor.matmul(
                                pyT,
                                w28_sl[d][:, 2 * k:2 * k + 2, :],
                                hT[:, 2 * k:2 * k + 2, :],
                                start=(k == 0),
                                stop=(k == KH // 2 - 1),
                                perf_mode=DR,
                            )
                    else:
                        for k in range(KH):
                            nc.tensor.matmul(
                                pyT,
                                w2b_sl[d][:, k, :],
                                hT[:, k, :],
                                start=(k == 0),
                                stop=(k == KH - 1),
                            )
                    # fused PSUM evacuation + gate broadcast multiply on DVE
                    nc.vector.tensor_mul(yt_blk[:, d, :], pyT, gb_sb)
                    if b == nblk - 1:
                        # last block: store each d-slice as its own DMA so
                        # the drain pipelines across queues behind the PE
                        # instead of one serial descriptor chain at the end
                        nc.sync.dma_start(
                            ytr3[:, d, t0b:t0b + tbs], yt_blk[:, d, :]
                        )
                if b != nblk - 1:
                    nc.sync.dma_start(ytr3[:, :, t0b:t0b + tbs], yt_blk)
    nc.compile()
    return nc


def _ensure_trace_hooks():
    # bass_utils' trace path (taken when BASS_TRACE=1 is set externally)
    # imports antenv.axon_hooks, which this image lacks. Shim it (and the
    # artifact upload, which needs a bucket) only when missing, so tracing
    # degrades gracefully instead of crashing.
    import sys
    import types

    try:
        import antenv.axon_hooks  # noqa: F401
        return
    except ImportError:
        pass
    try:
        import antenv

        mod = types.ModuleType("antenv.axon_hooks")
        state = {"hook": None}
        mod.set_axon_ntff_profile_hook = lambda h: state.__setitem__("hook", h)
        mod.get_axon_ntff_profile_hook = lambda: state["hook"]
        sys.modules["antenv.axon_hooks"] = mod
        antenv.axon_hooks = mod
        try:
            from trn_agent_boot.trn_boot import _ntff_profile_via_ctypes

            mod.set_axon_ntff_profile_hook(
                _ntff_profile_via_ctypes("/opt/axon/libaxon_pjrt.so")
            )
            import concourse.bass_utils as _bu

            _orig_upload = _bu.upload_artifacts

            def _safe_upload(tmpdir):
                try:
                    return _orig_upload(tmpdir)
                except Exception:
                    return f"local:{tmpdir}"

            _bu.upload_artifacts = _safe_upload
        except Exception:
            pass
    except Exception:
        pass


def _plan_cells(fe):
    """Bin-pack per-expert fp8 overflows fe into 8 uniform (s0, s1) cell
    pairs (one pair per core), single-expert cells, cells splittable across
    cores. Returns (sizes_c, assign) with assign[k][p] = (expert, lo, hi)
    meaning core k's C position p holds tokens [lo:hi) of that expert's fp8
    token list (None cell = all padding). Falls back to own-expert cells
    sized by the max overflow if no candidate packs."""
    import itertools
    E = len(fe)
    for sizes in [(432, 320), (448, 320), (448, 336), (464, 352),
                  (480, 368), (496, 384), (512, 416), (512, 512)]:
        s0, s1 = sizes
        # per-expert Pareto-minimal (a, b) cell counts
        opts = []
        for d in fe:
            o = [(a, b) for a in range(4) for b in range(4)
                 if a * s0 + b * s1 >= d]
            o = [c for c in o if not any(
                c2 != c and c2[0] <= c[0] and c2[1] <= c[1] for c2 in o)]
            opts.append(o)
        # DP with parent pointers over cumulative (na, nb) <= (8, 8)
        states = {(0, 0): []}
        ok = True
        for o in opts:
            nxt = {}
            for (na, nb), path in states.items():
                for a, b in o:
                    ns = (na + a, nb + b)
                    if ns[0] <= 8 and ns[1] <= 8 and ns not in nxt:
                        nxt[ns] = path + [(a, b)]
            if not nxt:
                ok = False
                break
            states = nxt
        if not ok:
            continue
        counts_ab = next(iter(states.values()))
        # materialize cells: position-0 cells then position-1 cells, each a
        # (expert, lo, hi) chunk of the expert's fp8 token list
        cells0, cells1 = [], []
        for e, (a, b) in enumerate(counts_ab):
            off = 0
            for _ in range(a):
                take = min(s0, max(0, fe[e] - off))
                cells0.append((e, off, off + take))
                off += take
            for _ in range(b):
                take = min(s1, max(0, fe[e] - off))
                cells1.append((e, off, off + take))
                off += take
        cells0 += [None] * (8 - len(cells0))
        cells1 += [None] * (8 - len(cells1))
        assign = [[cells0[k], cells1[k]] for k in range(8)]
        return [s0, s1], assign
    # fallback: own-expert cells, capacity = ceil16(max overflow)
    nf8 = max(256, -((-max(fe)) // 16) * 16)
    sizes_c = _split_blocks(nf8, mult16=True)
    assign = []
    for k in range(8):
        row, off = [], 0
        for tbs in sizes_c:
            take = min(tbs, max(0, fe[k] - off))
            row.append((k, off, off + take))
            off += take
        assign.append(row)
    return sizes_c, assign


def kernel(x, Wr, W1, b1, W2, b2):
    _ensure_trace_hooks()
    from concourse.bass_utils import run_bass_kernel_spmd

    f16 = np.float16
    e4m3 = ml_dtypes.float8_e4m3
    B, S, D = x.shape
    E, _, H = W1.shape
    N = B * S
    KD = D // P
    MJ = 16
    HJ = H // MJ
    xm = np.ascontiguousarray(x.reshape(N, D), dtype=np.float32)

    # --- host router (mirrors reference fp32 arithmetic; softmax is
    # monotonic so top-k on probs == top-k on logits, ties broken by index)
    logits = xm @ Wr
    mx = logits.max(axis=1, keepdims=True)
    ex = np.exp(logits - mx)
    probs = ex / ex.sum(axis=1, keepdims=True)
    top_i = np.argsort(-probs, axis=1, kind="stable")[:, :TOP_K]

    idx = [np.where((top_i == e).any(axis=1))[0] for e in range(E)]
    counts = np.array([len(i) for i in idx])
    cmax = int(counts.max())

    nbf = NBF_TARGET
    fe = [max(0, int(counts[e]) - nbf) for e in range(E)]
    sizes_a = _split_blocks(nbf, mult16=False)
    sizes_c, assign = _plan_cells(fe)
    nf8 = sum(sizes_c)
    ntot = nbf + nf8

    # --- dispatch: per expert, sort tokens by gate descending; largest nbf
    # gates -> tier A (bf16), rest -> tier C (fp8) + zero padding.
    xT = np.ascontiguousarray(xm.T).astype(f16)               # [D, N] fp16
    xT8 = np.ascontiguousarray((xm.T * SX)).astype(e4m3)      # [D, N] e4m3

    def pack_blocks(src, toks, ncap, sizes, dt):
        # [P, KD*ncap] block-packed SBUF layout
        xte = np.zeros((D, ncap), dtype=dt)
        xte[:, :len(toks)] = src[:, toks]
        xte3 = xte.reshape(KD, P, ncap).transpose(1, 0, 2)
        t0 = 0
        chunks = []
        for tbs in sizes:
            chunks.append(xte3[:, :, t0:t0 + tbs].reshape(P, -1))
            t0 += tbs
        return np.ascontiguousarray(np.concatenate(chunks, axis=1))

    # per-expert packed tensors (referenced by whichever core uses them)
    tok_a, tok_c = [], []
    w1bp, w18p_, w2bp, w28p_, b1p_ = [], [], [], [], []
    for e in range(E):
        ge = probs[idx[e], e]
        order = np.argsort(-ge, kind="stable")
        tok_a.append(idx[e][order[:nbf]])
        tok_c.append(idx[e][order[nbf:]])
        w1f = np.asarray(W1[e], dtype=np.float32)
        w2f = np.asarray(W2[e], dtype=np.float32)
        w1bp.append(np.ascontiguousarray(
            w1f.astype(f16).reshape(KD, P, MJ, HJ).transpose(1, 2, 0, 3)))
        w18p_.append(np.ascontiguousarray(
            (w1f * SW1).astype(e4m3).reshape(KD, P, MJ, HJ).transpose(1, 2, 0, 3)))
        w2bp.append(np.ascontiguousarray(
            w2f.astype(f16).reshape(H // P, P, KD, P).transpose(1, 2, 0, 3)))
        w28p_.append(np.ascontiguousarray(
            (w2f * SW2).astype(e4m3).reshape(H // P, P, KD, P).transpose(1, 2, 0, 3)))
        b1p_.append(np.ascontiguousarray(
            np.asarray(b1[e], dtype=np.float32).reshape(H // P, P).T))

    in_maps = []
    for k in range(E):
        ta = tok_a[k]
        # core k's C tokens: concatenation of its assigned cells' slices,
        # each padded to the cell size
        ctoks = []      # per cell: token array (len <= cell size)
        for p, cell in enumerate(assign[k]):
            if cell is None:
                ctoks.append(np.zeros(0, dtype=np.int64))
            else:
                e, lo, hi = cell
                ctoks.append(tok_c[e][lo:hi])
        # x dispatch: pack each cell into its block slot
        xc = np.zeros((D, ntot - nbf), dtype=e4m3)
        gfull = np.zeros((ntot,), dtype=np.float32)
        gfull[:len(ta)] = probs[ta, k]
        off = 0
        for p, tbs in enumerate(sizes_c):
            t = ctoks[p]
            if len(t):
                xc[:, off:off + len(t)] = xT8[:, t]
                e = assign[k][p][0]
                gfull[nbf + off:nbf + off + len(t)] = probs[t, e] / SW2
            off += tbs
        im = {
            "xtb": pack_blocks(xT, ta, nbf, sizes_a, f16),
            "xt8": pack_blocks(xc, np.arange(ntot - nbf), ntot - nbf,
                               sizes_c, e4m3),
            "w1b": w1bp[k],
            "w2b": w2bp[k],
            "gb": np.ascontiguousarray(
                np.broadcast_to(gfull, (P, ntot)).astype(f16)),
            "b1t": b1p_[k],
        }
        for p in range(len(sizes_c)):
            e = assign[k][p][0] if assign[k][p] is not None else 0
            im[f"w18_{p}"] = w18p_[e]
            im[f"w28_{p}"] = w28p_[e]
            im[f"b1c_{p}"] = b1p_[e]
        in_maps.append(im)

    key = (tuple(sizes_a), tuple(sizes_c), D, H)
    if key not in _program_cache:
        _program_cache[key] = _build_program(sizes_a, sizes_c, D, H)
    nc = _program_cache[key]

    res = run_bass_kernel_spmd(nc, in_maps, core_ids=list(range(NUM_EXPERTS)))

    # --- combine: per core, the A segment holds its own expert's tokens,
    # each C cell holds its assigned expert's token slice
    out = np.zeros((N, D), dtype=np.float32)
    b2f = np.asarray(b2, dtype=np.float32)
    for k in range(E):
        ytr = np.asarray(res.results[k]["ytr"]).astype(np.float32)
        segs = [(tok_a[k], k, 0)]
        off = nbf
        for p, tbs in enumerate(sizes_c):
            cell = assign[k][p]
            if cell is not None:
                e, lo, hi = cell
                segs.append((tok_c[e][lo:hi], e, off))
            off += tbs
        for toks, e, seg0 in segs:
            if not len(toks):
                continue
            ye = np.ascontiguousarray(ytr[:, seg0:seg0 + len(toks)].T)
            if b2f[e].any():
                ye = ye + probs[toks, e][:, None] * b2f[e]
            out[toks] += ye
    return out.reshape(B, S, D)


# revision 43
# speedup vs baseline: 1.0335x; 1.0041x over previous
# MoE (8 experts, top-2) on 8 TRN2 NeuronCores — expert-parallel, tiered
# precision.
#
# Host (numpy): router matmul + softmax + top-2 (mirrors the jax reference
# fp32 arithmetic), then per-expert dispatch into TWO tiers:
#   tier A (fp16):  the expert's nbf largest-gate tokens — fp16 matmuls
#                   (same PE rate as bf16, 8x finer mantissa: its error is
#                   ~0.9e-3 vs bf16's 3.4e-3, freeing budget for more fp8).
#   tier C (fp8):   the remaining tokens (smallest gates) + zero padding up
#                   to nf8 — e4m3 matmuls in DoubleRow perf mode (measured
#                   2.0x PE rate vs bf16 at FD>=300).
# (nbf, nf8) are uniform across cores so the single SPMD program fits every
# expert; capacity padding rides in the half-cost fp8 tier. Putting only
# low-gate tokens in fp8 keeps the quantization error a small fraction of
# the output norm (measured 1.83e-2 vs the 2e-2 gate; fp16-only is ~1e-3;
# the numpy e4m3 simulation in the dev harness predicts device error to
# ~1e-5, so the tier split was tuned against it).
#
# Device (per core, expert e), per token block:
#   mm1: hT = gelu(W1^T @ xT + b1)   (A: fp16, 8 K-chunks; C: e4m3 DoubleRow,
#        4 K-chunk-pairs, PSUM descaled 1/8192 inside the ACT op)
#   mm2: yT = (W2^T @ hT) * gate     (A: fp16; C: e4m3 DoubleRow with the
#        1/1024 W2 scale folded into the gate values — free)
# Both matmuls keep tokens on the moving/free axis so work scales with the
# exact token count. Gates and outputs are fp16 (halves the drain DMA).
#
# Weights are streamed per block (W1 slices, fp8 W2 d-slices) except fp16 W2
# which stays resident as 8 d-slices; SBUF peak ~26 MB, total DMA ~69
# MB/core — hidden under the ~390 us of PE time. Stream issue order is
# tuned so every tensor lands just before its first PE use (the in-order
# engine queues stall head-of-line otherwise).
#
# Shapes hardcoded for B=4, S=2048, D=1024, H=4096, E=8 (spec); the builder
# is parametric in the block lists (known only after routing).

import numpy as np
import ml_dtypes

NUM_EXPERTS = 8
TOP_K = 2
P = 128          # SBUF partitions
TB = 512         # token block (matmul moving free size)

# Tier sizes (tokens per expert): tuned offline on the seed-0 routing
# (counts 1932..2182, mean 2048). nbf + nf8 must cover the max expert load;
# the builder re-derives nf8 at runtime if routing ever exceeds it.
NBF_TARGET = 1344
NF8_TARGET = 848

# fp8 scales (powers of two; e4m3 max 240, scaled data max ~87)
SX = 16.0        # x
SW1 = 512.0      # W1  -> psum1 = 8192 * (x @ W1), descaled in the gelu ACT
SW2 = 1024.0     # W2  -> psum2 = 1024 * (h @ W2), folded into device gates

_program_cache = {}


def _split_blocks(n, mult16):
    """Split n tokens into blocks of <=TB, remainder last (block 0 must be
    full so its compute covers the next block's weight stream). For fp8
    (mult16) every block size must be a multiple of 16 (DoubleRow pair-slice
    stride) and at least 128 (DoubleRow is a net loss below FD=128), so a
    small remainder is rebalanced across the last two blocks."""
    if n == 0:
        return []
    sizes = [TB] * (n // TB)
    rem = n % TB
    if rem:
        sizes.append(rem)
    if mult16:
        assert n % 16 == 0
        if len(sizes) >= 2:
            # near-equal split: keeps every fp8 block >=256 (full DoubleRow
            # win) and shrinks the largest fp8 tile, which trims the h8/x8
            # SBUF footprint
            k = len(sizes)
            base = (n // k) // 16 * 16
            sizes = [base] * (k - 1) + [n - base * (k - 1)]
            assert sizes[-1] <= TB
    else:
        if len(sizes) >= 2 and sizes[-1] < 256:
            pair = sizes[-2] + sizes[-1]
            sizes[-2:] = [pair // 2, pair - pair // 2]
    return sizes


def _build_program(sizes_a, sizes_c, D, H):
    import concourse.mybir as mybir
    import concourse.tile as tile
    from concourse import bacc

    fh = mybir.dt.float16
    f8 = mybir.dt.float8e4
    f32 = mybir.dt.float32
    Gelu = mybir.ActivationFunctionType.Gelu_apprx_tanh
    DR = mybir.MatmulPerfMode.DoubleRow

    KD = D // P      # mm1 contraction chunks / mm2 output row chunks (8)
    KH = H // P      # mm1 output chunks / mm2 contraction chunks (32)
    MJ = 16          # W1 column-slice count (2 m-tiles per slice)
    HJ = H // MJ

    nbf = sum(sizes_a)
    nf8 = sum(sizes_c)
    ntot = nbf + nf8

    # Host-packed dram layouts (partition dim first, per-partition contiguous
    # runs >= 1KB so DMAs stream at full bandwidth):
    #   xtb : [P, KD*nbf] fp16    block-packed tier-A tokens
    #   xt8 : [P, KD*nf8] e4m3    block-packed tier-C tokens (x*16)
    #   w1b : [P, MJ, KD, HJ] fp16
    #   w18 : [P, MJ, KD, HJ] e4m3 (W1*512)
    #   w2b : [P, KD, KH, P]  fp16 (resident, d-major slices)
    #   w28 : [P, KD, KH, P]  e4m3 (W2*1024, d-major slices)
    #   gb  : [P, ntot] fp16      gates, tier-C segment pre-divided by 1024
    #   b1t : [P, KH] f32
    #   ytr : [D, ntot] fp16      transposed output, tier order A then C
    nc = bacc.Bacc(None, target_bir_lowering=False, debug=False)
    xtb = nc.declare_dram_parameter("xtb", [P, KD * max(nbf, 1)], fh, isOutput=False).ap()
    xt8 = nc.declare_dram_parameter("xt8", [P, KD * max(nf8, 1)], f8, isOutput=False).ap()
    w1b = nc.declare_dram_parameter("w1b", [P, MJ, KD, HJ], fh, isOutput=False).ap()
    w2b = nc.declare_dram_parameter("w2b", [P, KD, KH, P], fh, isOutput=False).ap()
    # per-C-position fp8 weights/bias: each C block position can be bound to
    # a DIFFERENT expert per core (the host bin-packs expert overflows into
    # uniform cells, cutting the fp8 capacity from ceil16(max overflow) to
    # near the mean overflow — worth ~10 us/core). Streaming volume is
    # unchanged: these were per-block streams already.
    w18P = [nc.declare_dram_parameter(f"w18_{p}", [P, MJ, KD, HJ], f8,
                                      isOutput=False).ap()
            for p in range(len(sizes_c))]
    w28P = [nc.declare_dram_parameter(f"w28_{p}", [P, KD, KH, P], f8,
                                      isOutput=False).ap()
            for p in range(len(sizes_c))]
    b1P = [nc.declare_dram_parameter(f"b1c_{p}", [P, KH], f32,
                                     isOutput=False).ap()
           for p in range(len(sizes_c))]
    gb = nc.declare_dram_parameter("gb", [P, ntot], fh, isOutput=False).ap()
    b1t = nc.declare_dram_parameter("b1t", [P, KH], f32, isOutput=False).ap()
    ytr = nc.declare_dram_parameter("ytr", [D, ntot], fh, isOutput=True).ap()
    # [P, KD, ntot] view of the [D, ntot] output for one-DMA-per-block stores
    ytr3 = ytr.rearrange("(d p) n -> p d n", p=P)

    # per-block metadata: (tbs, isf8, t0seg=offset within own tier, t0=global)
    blocks = []
    cpos_of = {}
    t0a = t0c = t0 = 0
    for tbs in sizes_a:
        blocks.append((tbs, False, t0a, t0))
        t0a += tbs
        t0 += tbs
    for p, tbs in enumerate(sizes_c):
        cpos_of[len(blocks)] = p
        blocks.append((tbs, True, t0c, t0))
        t0c += tbs
        t0 += tbs
    nblk = len(blocks)

    with tile.TileContext(nc) as tc:
        with (
            tc.tile_pool(name="wres", bufs=1) as wres,
            tc.tile_pool(name="wstr", bufs=4) as wstr,
            tc.tile_pool(name="data", bufs=2) as datap,
            tc.tile_pool(name="psum", bufs=4, space="PSUM") as psump,
        ):
            # merged pools (fewer pools -> shorter entry/exit barrier
            # handshake); rotation depth is set per tag via bufs=
            w1sp = w28p = wstr
            xpool = hpool = ypool = gbp = datap
            php = pyp = psump
            b1_sb = wres.tile([P, KH], f32, tag="b1sb")
            b1c_sb = [
                wres.tile([P, KH], f32, tag=f"b1c{p}", name=f"b1c{p}")
                for p in range(len(sizes_c))
            ]
            # resident bf16 W2 as 8 d-major slices: the tile scheduler
            # hoists a few mm2 matmuls high into the in-order PE queue, so
            # the first d-slice must land early (a monolithic 8.4 MB DMA
            # stalled the PE ~10-15 us).
            w2b_sl = [
                wres.tile([P, KH, P], fh, tag=f"w2bd{d}", name=f"w2bd{d}")
                for d in range(KD)
            ]

            # per-block input tiles, possibly issued one block ahead
            xt_tiles = {}
            gb_tiles = {}
            w1_tiles = {b: [None] * MJ for b in range(nblk)}

            def issue_x(b):
                tbs, isf8, t0seg, t0b = blocks[b]
                if isf8:
                    xt_blk = xpool.tile([P, KD, tbs], f8, tag="x8")
                    src, base = xt8, KD * t0seg
                else:
                    xt_blk = xpool.tile([P, KD, tbs], fh, tag="xt")
                    src, base = xtb, KD * t0seg
                if b == 0:
                    # block-0 x chunks issued from the DVE ring: SP's
                    # descriptor-issue rate (~0.6 us per DMACopy) serializes
                    # the ~12 startup descriptors otherwise, delaying the
                    # first matmul's data by several us
                    for j in range(KD // 2):
                        eng = nc.scalar if j < 2 else nc.gpsimd
                        eng.dma_start(
                            xt_blk[:, 2 * j:2 * j + 2, :],
                            src[:, base + 2 * j * tbs:
                                 base + (2 * j + 2) * tbs].rearrange(
                                "p (k c) -> p k c", k=2
                            ),
                        )
                else:
                    nc.sync.dma_start(
                        xt_blk,
                        src[:, base:base + KD * tbs].rearrange(
                            "p (k c) -> p k c", k=KD
                        ),
                    )
                xt_tiles[b] = xt_blk
                gb_sb = gbp.tile([P, tbs], fh, tag="gb")
                nc.sync.dma_start(gb_sb, gb[:, t0b:t0b + tbs])
                gb_tiles[b] = gb_sb

            def issue_w1(b, js):
                _, isf8, _, _ = blocks[b]
                wsrc, wdt, wtag = ((w18P[cpos_of[b]], f8, "w18s") if isf8
                                   else (w1b, fh, "w1bs"))
                for j in js:
                    if w1_tiles[b][j] is not None:
                        continue
                    ws = w1sp.tile([P, KD, HJ], wdt, tag=wtag)
                    nc.sync.dma_start(ws, wsrc[:, j, :, :])
                    w1_tiles[b][j] = ws

            # PE pstate warmup: the engine runs its first ~3 us at a reduced
            # p-state (first real m-tile measured 427-634 ns/matmul instead
            # of 216). Zero matmuls on memset tiles ramp it up during the
            # startup DMA window; they retire before the first x/W1 slice
            # lands (~11 us), so they never delay real work.
            warm_l = wres.tile([P, P], fh, tag="warm_l")
            warm_r = wres.tile([P, TB // 2], fh, tag="warm_r")
            nc.vector.memset(warm_l, 0.0)
            nc.vector.memset(warm_r, 0.0)
            for _ in range(24):
                pw = php.tile([P, TB // 2], f32, tag="ph")
                nc.tensor.matmul(pw, warm_l, warm_r, start=True, stop=True)

            issue_w1(0, range(0, 1))
            issue_x(0)
            nc.gpsimd.dma_start(b1_sb, b1t)
            for p in range(len(sizes_c)):
                nc.gpsimd.dma_start(b1c_sb[p], b1P[p])
            issue_w1(0, range(1, 4))
            # d0 early: the scheduler hoists a few mm2-d0 matmuls between
            # mm1 m-tiles ~6-7, so d0 must land by ~25 us. d1..d7 are only
            # needed at the real mm2 (~105 us) — after the W1 stream.
            nc.sync.dma_start(w2b_sl[0], w2b[:, 0, :, :])
            issue_w1(0, range(4, MJ))
            for d in range(1, KD):
                nc.sync.dma_start(w2b_sl[d], w2b[:, d, :, :])

            for b, (tbs, isf8, t0seg, t0b) in enumerate(blocks):
                # finish this block's input streams (x/gb/first W1 slices
                # were issued during the previous block). For fp8 blocks the
                # W2 d-slices are interleaved into the W1 stream by need
                # time (W1 slice j feeds m-tiles 2j at ~j*1.05 us; W2 slice
                # d feeds mm2 at ~mm1_end + d*2.6 us) — issuing all W1
                # first made mm2's first d-slices arrive late (~3.5 us PE
                # stall per fp8 block).
                w28_sl = []
                if isf8:
                    w28src = w28P[cpos_of[b]]

                    def issue_w28(dds):
                        for dd in dds:
                            w2s = w28p.tile([P, KH, P], f8, tag="w28s", bufs=8)
                            nc.sync.dma_start(w2s, w28src[:, dd, :, :])
                            w28_sl.append(w2s)
                    issue_w1(b, range(0, 10))
                    issue_w28(range(0, 2))
                    issue_w1(b, range(10, MJ))
                    issue_w28(range(2, KD))
                else:
                    issue_w1(b, range(MJ))
                # prefetch the next block's x, gates, and first W1 slices so
                # the A->C phase switch doesn't wait behind this block's
                # 8.4 MB W1 stream
                if b + 1 < nblk:
                    issue_x(b + 1)
                    issue_w1(b + 1, range(0, 3))

                xt_blk = xt_tiles.pop(b)
                gb_sb = gb_tiles.pop(b)
                w1_sl = w1_tiles.pop(b)

                # --- mm1: hT[m] = gelu(W1_chunk^T @ xT + b1) -> [P, tbs]
                hT = hpool.tile([P, KH, tbs], f8 if isf8 else fh,
                                tag="h8" if isf8 else "hT", bufs=1)
                for m in range(KH):
                    ph = php.tile([P, tbs], f32, tag="ph")
                    mj, mo = divmod(m, HJ // P)
                    if isf8:
                        for k in range(KD // 2):
                            nc.tensor.matmul(
                                ph,
                                w1_sl[mj][:, 2 * k:2 * k + 2, mo * P:(mo + 1) * P],
                                xt_blk[:, 2 * k:2 * k + 2, :],
                                start=(k == 0),
                                stop=(k == KD // 2 - 1),
                                perf_mode=DR,
                            )
                        nc.scalar.activation(
                            hT[:, m, :], ph, Gelu,
                            bias=b1c_sb[cpos_of[b]][:, m:m + 1],
                            scale=1.0 / (SX * SW1),
                        )
                    else:
                        for k in range(KD):
                            nc.tensor.matmul(
                                ph,
                                w1_sl[mj][:, k, mo * P:(mo + 1) * P],
                                xt_blk[:, k, :],
                                start=(k == 0),
                                stop=(k == KD - 1),
                            )
                        nc.scalar.activation(
                            hT[:, m, :], ph, Gelu, bias=b1_sb[:, m:m + 1]
                        )

                # --- mm2: yT[d] = (W2_chunk^T @ hT) * gate, one output DMA
                # per block (8 small stores per block cost ~0.6 us of SP
                # issue each and serialized the drain tail)
                yt_blk = ypool.tile([P, KD, tbs], fh, tag="yt")
                for d in range(KD):
                    pyT = pyp.tile([P, tbs], f32, tag="py")
                    if isf8:
                        for k in range(KH // 2):
                            nc.tensor.matmul(
                                pyT,
                                w28_sl[d][:, 2 * k:2 * k + 2, :],
                                hT[:, 2 * k:2 * k + 2, :],
                                start=(k == 0),
                                stop=(k == KH // 2 - 1),
                                perf_mode=DR,
                            )
                    else:
                        for k in range(KH):
                            nc.tensor.matmul(
                                pyT,
                                w2b_sl[d][:, k, :],
                                hT[:, k, :],
                                start=(k == 0),
                                stop=(k == KH - 1),
                            )
                    # fused PSUM evacuation + gate broadcast multiply on DVE
                    nc.vector.tensor_mul(yt_blk[:, d, :], pyT, gb_sb)
                    if b == nblk - 1:
                        # last block: store each d-slice as its own DMA so
                        # the drain pipelines across queues behind the PE
                        # instead of one serial descriptor chain at the end
                        nc.sync.dma_start(
                            ytr3[:, d, t0b:t0b + tbs], yt_blk[:, d, :]
                        )
                if b != nblk - 1:
                    nc.sync.dma_start(ytr3[:, :, t0b:t0b + tbs], yt_blk)
    nc.compile()
    return nc


def _ensure_trace_hooks():
    # bass_utils' trace path (taken when BASS_TRACE=1 is set externally)
    # imports antenv.axon_hooks, which this image lacks. Shim it (and the
    # artifact upload, which needs a bucket) only when missing, so tracing
    # degrades gracefully instead of crashing.
    import sys
    import types

    try:
        import antenv.axon_hooks  # noqa: F401
        return
    except ImportError:
        pass
    try:
        import antenv

        mod = types.ModuleType("antenv.axon_hooks")
        state = {"hook": None}
        mod.set_axon_ntff_profile_hook = lambda h: state.__setitem__("hook", h)
        mod.get_axon_ntff_profile_hook = lambda: state["hook"]
        sys.modules["antenv.axon_hooks"] = mod
        antenv.axon_hooks = mod
        try:
            from trn_agent_boot.trn_boot import _ntff_profile_via_ctypes

            mod.set_axon_ntff_profile_hook(
                _ntff_profile_via_ctypes("/opt/axon/libaxon_pjrt.so")
            )
            import concourse.bass_utils as _bu

            _orig_upload = _bu.upload_artifacts

            def _safe_upload(tmpdir):
                try:
                    return _orig_upload(tmpdir)
                except Exception:
                    return f"local:{tmpdir}"

            _bu.upload_artifacts = _safe_upload
        except Exception:
            pass
    except Exception:
        pass


def _plan_cells(fe):
    """Bin-pack per-expert fp8 overflows fe into 8 uniform (s0, s1) cell
    pairs (one pair per core), single-expert cells, cells splittable across
    cores. Returns (sizes_c, assign) with assign[k][p] = (expert, lo, hi)
    meaning core k's C position p holds tokens [lo:hi) of that expert's fp8
    token list (None cell = all padding). Falls back to own-expert cells
    sized by the max overflow if no candidate packs."""
    import itertools
    E = len(fe)
    for sizes in [(432, 320), (448, 320), (448, 336), (464, 352),
                  (480, 368), (496, 384), (512, 416), (512, 512)]:
        s0, s1 = sizes
        # per-expert Pareto-minimal (a, b) cell counts
        opts = []
        for d in fe:
            o = [(a, b) for a in range(4) for b in range(4)
                 if a * s0 + b * s1 >= d]
            o = [c for c in o if not any(
                c2 != c and c2[0] <= c[0] and c2[1] <= c[1] for c2 in o)]
            opts.append(o)
        # DP with parent pointers over cumulative (na, nb) <= (8, 8)
        states = {(0, 0): []}
        ok = True
        for o in opts:
            nxt = {}
            for (na, nb), path in states.items():
                for a, b in o:
                    ns = (na + a, nb + b)
                    if ns[0] <= 8 and ns[1] <= 8 and ns not in nxt:
                        nxt[ns] = path + [(a, b)]
            if not nxt:
                ok = False
                break
            states = nxt
        if not ok:
            continue
        counts_ab = next(iter(states.values()))
        # materialize cells: position-0 cells then position-1 cells, each a
        # (expert, lo, hi) chunk of the expert's fp8 token list
        cells0, cells1 = [], []
        for e, (a, b) in enumerate(counts_ab):
            off = 0
            for _ in range(a):
                take = min(s0, max(0, fe[e] - off))
                cells0.append((e, off, off + take))
                off += take
            for _ in range(b):
                take = min(s1, max(0, fe[e] - off))
                cells1.append((e, off, off + take))
                off += take
        cells0 += [None] * (8 - len(cells0))
        cells1 += [None] * (8 - len(cells1))
        assign = [[cells0[k], cells1[k]] for k in range(8)]
        return [s0, s1], assign
    # fallback: own-expert cells, capacity = ceil16(max overflow)
    nf8 = max(256, -((-max(fe)) // 16) * 16)
    sizes_c = _split_blocks(nf8, mult16=True)
    assign = []
    for k in range(8):
        row, off = [], 0
        for tbs in sizes_c:
            take = min(tbs, max(0, fe[k] - off))
            row.append((k, off, off + take))
            off += take
        assign.append(row)
    return sizes_c, assign


def kernel(x, Wr, W1, b1, W2, b2):
    _ensure_trace_hooks()
    from concourse.bass_utils import run_bass_kernel_spmd

    f16 = np.float16
    e4m3 = ml_dtypes.float8_e4m3
    B, S, D = x.shape
    E, _, H = W1.shape
    N = B * S
    KD = D // P
    MJ = 16
    HJ = H // MJ
    xm = np.ascontiguousarray(x.reshape(N, D), dtype=np.float32)

    # --- host router (mirrors reference fp32 arithmetic; softmax is
    # monotonic so top-k on probs == top-k on logits, ties broken by index)
    logits = xm @ Wr
    mx = logits.max(axis=1, keepdims=True)
    ex = np.exp(logits - mx)
    probs = ex / ex.sum(axis=1, keepdims=True)
    top_i = np.argsort(-probs, axis=1, kind="stable")[:, :TOP_K]

    idx = [np.where((top_i == e).any(axis=1))[0] for e in range(E)]
    counts = np.array([len(i) for i in idx])
    cmax = int(counts.max())

    nbf = NBF_TARGET
    fe = [max(0, int(counts[e]) - nbf) for e in range(E)]
    sizes_a = _split_blocks(nbf, mult16=False)
    sizes_c, assign = _plan_cells(fe)
    nf8 = sum(sizes_c)
    ntot = nbf + nf8

    # --- dispatch: per expert, sort tokens by gate descending; largest nbf
    # gates -> tier A (bf16), rest -> tier C (fp8) + zero padding.
    xT = np.ascontiguousarray(xm.T).astype(f16)               # [D, N] fp16
    xT8 = np.ascontiguousarray((xm.T * SX)).astype(e4m3)      # [D, N] e4m3

    def pack_blocks(src, toks, ncap, sizes, dt):
        # [P, KD*ncap] block-packed SBUF layout
        xte = np.zeros((D, ncap), dtype=dt)
        xte[:, :len(toks)] = src[:, toks]
        xte3 = xte.reshape(KD, P, ncap).transpose(1, 0, 2)
        t0 = 0
        chunks = []
        for tbs in sizes:
            chunks.append(xte3[:, :, t0:t0 + tbs].reshape(P, -1))
            t0 += tbs
        return np.ascontiguousarray(np.concatenate(chunks, axis=1))

    # per-expert packed tensors (referenced by whichever core uses them)
    tok_a, tok_c = [], []
    w1bp, w18p_, w2bp, w28p_, b1p_ = [], [], [], [], []
    for e in range(E):
        ge = probs[idx[e], e]
        order = np.argsort(-ge, kind="stable")
        tok_a.append(idx[e][order[:nbf]])
        tok_c.append(idx[e][order[nbf:]])
        w1f = np.asarray(W1[e], dtype=np.float32)
        w2f = np.asarray(W2[e], dtype=np.float32)
        w1bp.append(np.ascontiguousarray(
            w1f.astype(f16).reshape(KD, P, MJ, HJ).transpose(1, 2, 0, 3)))
        w18p_.append(np.ascontiguousarray(
            (w1f * SW1).astype(e4m3).reshape(KD, P, MJ, HJ).transpose(1, 2, 0, 3)))
        w2bp.append(np.ascontiguousarray(
            w2f.astype(f16).reshape(H // P, P, KD, P).transpose(1, 2, 0, 3)))
        w28p_.append(np.ascontiguousarray(
            (w2f * SW2).astype(e4m3).reshape(H // P, P, KD, P).transpose(1, 2, 0, 3)))
        b1p_.append(np.ascontiguousarray(
            np.asarray(b1[e], dtype=np.float32).reshape(H // P, P).T))

    in_maps = []
    for k in range(E):
        ta = tok_a[k]
        # core k's C tokens: concatenation of its assigned cells' slices,
        # each padded to the cell size
        ctoks = []      # per cell: token array (len <= cell size)
        for p, cell in enumerate(assign[k]):
            if cell is None:
                ctoks.append(np.zeros(0, dtype=np.int64))
            else:
                e, lo, hi = cell
                ctoks.append(tok_c[e][lo:hi])
        # x dispatch: pack each cell into its block slot
        xc = np.zeros((D, ntot - nbf), dtype=e4m3)
        gfull = np.zeros((ntot,), dtype=np.float32)
        gfull[:len(ta)] = probs[ta, k]
        off = 0
        for p, tbs in enumerate(sizes_c):
            t = ctoks[p]
            if len(t):
                xc[:, off:off + len(t)] = xT8[:, t]
                e = assign[k][p][0]
                gfull[nbf + off:nbf + off + len(t)] = probs[t, e] / SW2
            off += tbs
        im = {
            "xtb": pack_blocks(xT, ta, nbf, sizes_a, f16),
            "xt8": pack_blocks(xc, np.arange(ntot - nbf), ntot - nbf,
                               sizes_c, e4m3),
            "w1b": w1bp[k],
            "w2b": w2bp[k],
            "gb": np.ascontiguousarray(
                np.broadcast_to(gfull, (P, ntot)).astype(f16)),
            "b1t": b1p_[k],
        }
        for p in range(len(sizes_c)):
            e = assign[k][p][0] if assign[k][p] is not None else 0
            im[f"w18_{p}"] = w18p_[e]
            im[f"w28_{p}"] = w28p_[e]
            im[f"b1c_{p}"] = b1p_[e]
        in_maps.append(im)

    key = (tuple(sizes_a), tuple(sizes_c), D, H)
    if key not in _program_cache:
        _program_cache[key] = _build_program(sizes_a, sizes_c, D, H)
    nc = _program_cache[key]

    res = run_bass_kernel_spmd(nc, in_maps, core_ids=list(range(NUM_EXPERTS)))

    # --- combine: per core, the A segment holds its own expert's tokens,
    # each C cell holds its assigned expert's token slice
    out = np.zeros((N, D), dtype=np.float32)
    b2f = np.asarray(b2, dtype=np.float32)
    for k in range(E):
        ytr = np.asarray(res.results[k]["ytr"]).astype(np.float32)
        segs = [(tok_a[k], k, 0)]
        off = nbf
        for p, tbs in enumerate(sizes_c):
            cell = assign[k][p]
            if cell is not None:
                e, lo, hi = cell
                segs.append((tok_c[e][lo:hi], e, off))
            off += tbs
        for toks, e, seg0 in segs:
            if not len(toks):
                continue
            ye = np.ascontiguousarray(ytr[:, seg0:seg0 + len(toks)].T)
            if b2f[e].any():
                ye = ye + probs[toks, e][:, None] * b2f[e]
            out[toks] += ye
    return out.reshape(B, S, D)
